# revision 1
# baseline (speedup 1.0000x reference)
"""Trainium2 Bass kernel for the LocalGNOBlock (windowed GNN message passing).

Math restructuring (vs the naive 12x full MLP evaluations):
  msg first layer is linear over concat([h_i, h_j, dc]):
      z_d[i] = (A - C)[i] + (B + C)[i+d] + b1,  d in {+-1..+-6}
  where A = h @ W1a, B = h @ W1b, C = coord x w1c (rank-1).
  The second msg layer is summed over edges BEFORE the matmul:
      agg_pre = (sum_d silu(z_d)) @ W2
  Aggregate divide-by-count folds into W2 (interior count == 12) with a
  6-column fixup at each sequence end.  LayerNorm stats are computed with
  ones-vector matmuls (channel dim lives on partitions); the normalize uses
  rank-1 broadcast grids P1 = g x r, P2 = g x (mu*r) - b x 1 built on the PE.

Sharding: batch dim B=8 -> one batch element per NeuronCore (no halo needed).
Host pre/post: transpose h -> [128, N] per core (channel-major), transpose the
[128, N] output back.  Device time is what counts; host transposes are cheap.
"""

import numpy as np

K = 6
HID = 128
N = 16384
B = 8
EPS = 1e-5
T = 512                 # token chunk (matmul + elementwise granularity)
NCH = N // T            # 32 chunks
OFF0 = 8                # D_full column of token 0 (even, for bf16 alignment)
NCOL = N + 2 * OFF0     # D_full width

# offsets ordered in 4 stride-2 groups: (even uses D_A, odd uses D_B)
NEG_EVEN = [-6, -4, -2]
NEG_ODD = [-5, -3, -1]
POS_ODD = [1, 3, 5]
POS_EVEN = [2, 4, 6]
SEG_ORDER = NEG_EVEN + NEG_ODD + POS_ODD + POS_EVEN  # 12 segments in Z

_compiled = None


def _build_bass(dt_act):
    import concourse.bacc as bacc
    import concourse.bass as bass
    import concourse.tile as tile
    from concourse import mybir

    f32 = mybir.dt.float32
    DT = dt_act

    nc = bacc.Bacc("TRN2", target_bir_lowering=False, debug=False)

    # ---- DRAM I/O ----
    hT = nc.dram_tensor("hT", [HID, N], DT, kind="ExternalInput")
    coordR = nc.dram_tensor("coordR", [1, N], DT, kind="ExternalInput")
    W1a = nc.dram_tensor("W1a", [HID, HID], DT, kind="ExternalInput")
    W1b = nc.dram_tensor("W1b", [HID, HID], DT, kind="ExternalInput")
    w1c = nc.dram_tensor("w1c", [1, HID], DT, kind="ExternalInput")      # +w1c
    w1cn = nc.dram_tensor("w1cn", [1, HID], DT, kind="ExternalInput")    # -w1c
    W2s = nc.dram_tensor("W2s", [HID, HID], DT, kind="ExternalInput")     # W2/12
    U1a = nc.dram_tensor("U1a", [HID, HID], DT, kind="ExternalInput")
    U1b = nc.dram_tensor("U1b", [HID, HID], DT, kind="ExternalInput")
    U2 = nc.dram_tensor("U2", [HID, HID], DT, kind="ExternalInput")
    b1c = nc.dram_tensor("b1c", [HID, 1], f32, kind="ExternalInput")      # msg_b1
    buc = nc.dram_tensor("buc", [HID, 1], f32, kind="ExternalInput")      # upd_b1 + b2@U1b
    b2u = nc.dram_tensor("b2u", [1, HID], DT, kind="ExternalInput")      # upd_b2 row
    g_row = nc.dram_tensor("g_row", [1, HID], DT, kind="ExternalInput")  # ln_g
    nb_row = nc.dram_tensor("nb_row", [1, HID], DT, kind="ExternalInput")  # -ln_b
    ident = nc.dram_tensor("ident", [HID, HID], DT, kind="ExternalInput")
    ones_col = nc.dram_tensor("ones_col", [HID, 1], DT, kind="ExternalInput")  # 1/128
    fixf = nc.dram_tensor("fixf", [1, K], f32, kind="ExternalInput")      # 12/count head
    fixl = nc.dram_tensor("fixl", [1, K], f32, kind="ExternalInput")      # 12/count tail
    # band-select matrix: column 63 = 1/128, else 0 (stats row packing)
    selb = nc.dram_tensor("selb", [HID, 2 * 2 * NCH - 1], DT, kind="ExternalInput")
    outT = nc.dram_tensor("outT", [HID, N], f32, kind="ExternalOutput")

    Silu = mybir.ActivationFunctionType.Silu
    Sqrt = mybir.ActivationFunctionType.Sqrt

    with tile.TileContext(nc) as tc:
        with (
            tc.tile_pool(name="singles", bufs=1) as singles,
            tc.tile_pool(name="big", bufs=1) as big,
            tc.tile_pool(name="work", bufs=3) as work,
            tc.tile_pool(name="zpool", bufs=2) as zpool,
            tc.tile_pool(name="opool", bufs=3) as opool,
            tc.tile_pool(name="psA", bufs=1, space="PSUM") as psA,
            tc.tile_pool(name="psB", bufs=1, space="PSUM") as psB,
            tc.tile_pool(name="psS", bufs=1, space="PSUM") as psS,
        ):
            # ---- constants into SBUF ----
            sW1a = singles.tile([HID, HID], DT)
            sW1b = singles.tile([HID, HID], DT)
            sW2s = singles.tile([HID, HID], DT)
            sU1a = singles.tile([HID, HID], DT)
            sU1b = singles.tile([HID, HID], DT)
            sU2 = singles.tile([HID, HID], DT)
            sIdent = singles.tile([HID, HID], DT)
            for sb, dr in [(sW1a, W1a), (sW1b, W1b), (sW2s, W2s),
                           (sU1a, U1a), (sU1b, U1b), (sU2, U2), (sIdent, ident)]:
                nc.sync.dma_start(out=sb, in_=dr[:, :])
            sw1c = singles.tile([1, HID], DT)
            sw1cn = singles.tile([1, HID], DT)
            sb2u = singles.tile([1, HID], DT)
            sg = singles.tile([1, HID], DT)
            snb = singles.tile([1, HID], DT)
            for sb, dr in [(sw1c, w1c), (sw1cn, w1cn), (sb2u, b2u),
                           (sg, g_row), (snb, nb_row)]:
                nc.sync.dma_start(out=sb, in_=dr[:, :])
            sb1 = singles.tile([HID, 1], f32)
            sbu = singles.tile([HID, 1], f32)
            sones = singles.tile([HID, 1], DT)
            nc.sync.dma_start(out=sb1, in_=b1c[:, :])
            nc.sync.dma_start(out=sbu, in_=buc[:, :])
            nc.sync.dma_start(out=sones, in_=ones_col[:, :])
            # broadcast [1,6] -> [128,6] fix tiles
            sfixf = singles.tile([HID, K], f32)
            sfixl = singles.tile([HID, K], f32)
            def bcast_rows(dr):
                a = dr[0:1, :]
                return bass.AP(tensor=a.tensor, offset=a.offset,
                               ap=[[0, HID]] + list(a.ap[1:]))

            nc.gpsimd.dma_start(out=sfixf, in_=bcast_rows(fixf))
            nc.gpsimd.dma_start(out=sfixl, in_=bcast_rows(fixl))
            sones_row = singles.tile([1, T], DT)
            nc.vector.memset(sones_row, 1.0)
            ssel = singles.tile([HID, 2 * 2 * NCH - 1], DT)
            nc.sync.dma_start(out=ssel, in_=selb[:, :])

            # ---- big persistent buffers ----
            D_A = big.tile([HID, NCOL], DT)      # token j at col OFF0 + j
            D_B = big.tile([HID, NCOL], DT)      # token j at col OFF0 + 1 + j
            x_full = big.tile([HID, N], DT)
            # zero halo columns of D so boundary silu stays finite
            nc.vector.memset(D_A[:, 0:OFF0], 0.0)
            nc.vector.memset(D_A[:, OFF0 + N:NCOL], 0.0)
            nc.vector.memset(D_B[:, 0:OFF0 + 1], 0.0)
            nc.vector.memset(D_B[:, OFF0 + 1 + N:NCOL], 0.0)

            # LN stats: rows [0:32] = E[x]/chunk, [32:64] = E[x^2]/chunk
            st_ps = psS.tile([2 * NCH, T], f32)

            hts = {}
            crd = {}

            def load_chunk(c):
                ht = work.tile([HID, T], DT, tag="ht")
                nc.sync.dma_start(out=ht, in_=hT[:, c * T:(c + 1) * T])
                co = work.tile([1, T], DT, tag="co")
                nc.sync.dma_start(out=co, in_=coordR[:, c * T:(c + 1) * T])
                hts[c] = ht
                crd[c] = co

            def phase_a(c):
                # D chunk = W1b.T @ h  +  w1c x coord   (PSUM accumulate)
                d_ps = psA.tile([HID, T], f32, tag="de", bufs=2)
                nc.tensor.matmul(d_ps, sW1b, hts[c], start=True, stop=False)
                nc.tensor.matmul(d_ps, sw1c, crd[c], start=False, stop=True)
                col = OFF0 + c * T
                nc.vector.tensor_copy(D_A[:, col:col + T], d_ps)
                nc.gpsimd.tensor_copy(
                    out=D_B[:, col + 1:col + 1 + T], in_=D_A[:, col:col + T])

            def seg_in1(tile_ap, col):
                # [128, 3, T] AP over D with outer column-stride 2
                s = tile_ap[:, col:col + T]
                return bass.AP(tensor=s.tensor, offset=s.offset,
                               ap=[s.ap[0], [2, 3], [1, T]])

            def phase_b(t):
                ht, co = hts[t], crd[t]
                # E chunk = W1a.T @ h - w1c x coord
                e_ps = psA.tile([HID, T], f32, tag="de", bufs=2)
                nc.tensor.matmul(e_ps, sW1a, ht, start=True, stop=False)
                nc.tensor.matmul(e_ps, sw1cn, co, start=False, stop=True)
                e_sb = work.tile([HID, T], DT, tag="esb")
                nc.vector.tensor_copy(e_sb, e_ps)

                # Z: 12 segments of E + shifted D, 4 stride-2 groups
                z = zpool.tile([HID, 12 * T], DT, tag="z")
                zv = z.rearrange("p (s t) -> p s t", t=T)
                e_b = bass.AP(tensor=e_sb.tensor, offset=e_sb.offset,
                              ap=[e_sb.ap[0], [0, 3], [1, T]])
                base = t * T
                groups = [
                    (D_A, OFF0 + base + NEG_EVEN[0]),
                    (D_B, OFF0 + 1 + base + NEG_ODD[0]),
                    (D_B, OFF0 + 1 + base + POS_ODD[0]),
                    (D_A, OFF0 + base + POS_EVEN[0]),
                ]
                for gi, (dbuf, col) in enumerate(groups):
                    nc.vector.tensor_tensor(
                        out=zv[:, 3 * gi:3 * gi + 3, :],
                        in0=e_b, in1=seg_in1(dbuf, col),
                        op=mybir.AluOpType.add)

                # silu over all 12 segments at once (bias = msg_b1)
                nc.scalar.activation(z, z, Silu, bias=sb1, scale=1.0)

                # zero invalid boundary columns (torn edges of the sequence)
                if t == 0:
                    for s, d in enumerate(SEG_ORDER):
                        if d < 0:
                            nc.vector.memset(zv[:, s, 0:-d], 0.0)
                if t == NCH - 1:
                    for s, d in enumerate(SEG_ORDER):
                        if d > 0:
                            nc.vector.memset(zv[:, s, T - d:T], 0.0)

                # agg_pre = sum_s silu(z_s) @ W2s   (PSUM accumulation)
                a_ps = psB.tile([HID, T], f32, tag="agg")
                for s in range(12):
                    nc.tensor.matmul(a_ps, sW2s, zv[:, s, :],
                                     start=(s == 0), stop=(s == 11))
                agg = work.tile([HID, T], DT, tag="agg_sb")
                nc.vector.tensor_copy(agg, a_ps)
                if t == 0:
                    nc.vector.tensor_tensor(out=agg[:, 0:K], in0=a_ps[:, 0:K],
                                            in1=sfixf, op=mybir.AluOpType.mult)
                if t == NCH - 1:
                    nc.vector.tensor_tensor(out=agg[:, T - K:T],
                                            in0=a_ps[:, T - K:T],
                                            in1=sfixl, op=mybir.AluOpType.mult)

                # update MLP
                u_ps = psA.tile([HID, T], f32, tag="upd", bufs=2)
                nc.tensor.matmul(u_ps, sU1a, ht, start=True, stop=False)
                nc.tensor.matmul(u_ps, sU1b, agg, start=False, stop=True)
                s2 = work.tile([HID, T], DT, tag="s2")
                nc.scalar.activation(s2, u_ps, Silu, bias=sbu, scale=1.0)

                # x = h + silu@U2 + b2u  (all accumulated in PSUM)
                x_ps = psA.tile([HID, T], f32, tag="xps", bufs=2)
                nc.tensor.matmul(x_ps, sU2, s2, start=True, stop=False)
                nc.tensor.matmul(x_ps, sb2u, sones_row, start=False, stop=False)
                nc.tensor.matmul(x_ps, sIdent, ht, start=False, stop=True)
                x_sb = x_full[:, base:base + T]
                nc.vector.tensor_copy(x_sb, x_ps)
                x2 = work.tile([HID, T], DT, tag="x2")
                nc.vector.tensor_tensor(out=x2, in0=x_sb, in1=x_sb,
                                        op=mybir.AluOpType.mult)
                # LN stats rows: band-select lhsT packs E[x] into psum row t
                # and E[x^2] into row NCH+t of one accumulating [64,T] bank
                hot = 2 * NCH - 1
                nc.tensor.matmul(st_ps[:, :], ssel[:, hot - t:hot - t + 2 * NCH],
                                 x_sb, start=(t == 0), stop=False)
                nc.tensor.matmul(st_ps[:, :],
                                 ssel[:, hot - NCH - t:hot - t + NCH],
                                 x2, start=False, stop=(t == NCH - 1))

            # ---------------- pass 1 ----------------
            load_chunk(0)
            for c in range(NCH + 1):
                if c < NCH:
                    if c + 1 < NCH:
                        load_chunk(c + 1)
                    phase_a(c)
                if c >= 1:
                    phase_b(c - 1)

            # ---------------- LN stats math ----------------
            r_sb = big.tile([NCH, T], DT)       # rstd per token
            u_sb = big.tile([NCH, T], DT)       # mu * rstd per token
            ex_sb = work.tile([NCH, T], f32, tag="ex")
            nc.vector.tensor_copy(ex_sb, st_ps[0:NCH, :])
            t1 = work.tile([NCH, T], f32, tag="t1")
            nc.vector.tensor_tensor(out=t1, in0=ex_sb, in1=ex_sb,
                                    op=mybir.AluOpType.mult)
            var = work.tile([NCH, T], f32, tag="var")
            nc.vector.tensor_tensor(out=var, in0=st_ps[NCH:2 * NCH, :], in1=t1,
                                    op=mybir.AluOpType.subtract)
            seps = singles.tile([NCH, 1], f32)
            nc.vector.memset(seps, float(EPS))
            nc.scalar.activation(var, var, Sqrt, bias=seps, scale=1.0)
            with nc.allow_low_precision(reason="rstd rows feed fp16 matmuls"):
                nc.vector.reciprocal(out=r_sb, in_=var)
            nc.vector.tensor_tensor(out=u_sb, in0=ex_sb,
                                    in1=r_sb, op=mybir.AluOpType.mult)
            # ---------------- pass 2: normalize ----------------
            # K=1 matmul rhs must start at partition 0: DMA each row down
            for t in range(NCH):
                base = t * T
                rr = work.tile([1, T], DT, tag="rr")
                nc.sync.dma_start(out=rr, in_=r_sb[t:t + 1, :])
                uu = work.tile([1, T], DT, tag="uu")
                nc.sync.dma_start(out=uu, in_=u_sb[t:t + 1, :])
                p1 = psA.tile([HID, T], f32, tag="upd", bufs=2)
                nc.tensor.matmul(p1, sg, rr, start=True, stop=True)
                p2 = psA.tile([HID, T], f32, tag="xps", bufs=2)
                nc.tensor.matmul(p2, sg, uu, start=True, stop=False)
                nc.tensor.matmul(p2, snb, sones_row, start=False, stop=True)
                o = opool.tile([HID, T], f32, tag="o")
                nc.vector.tensor_tensor(out=o, in0=x_full[:, base:base + T],
                                        in1=p1, op=mybir.AluOpType.mult)
                nc.vector.tensor_tensor(out=o, in0=o, in1=p2,
                                        op=mybir.AluOpType.subtract)
                nc.sync.dma_start(out=outT[:, base:base + T], in_=o)

    nc.compile()
    return nc


def _get_compiled(dt_name):
    global _compiled
    if _compiled is None:
        from concourse import mybir
        dt = {"bf16": mybir.dt.bfloat16, "fp16": mybir.dt.float16, "fp32": mybir.dt.float32}[dt_name]
        _compiled = _build_bass(dt)
    return _compiled


DT_NAME = "fp16"


def _sel_band(act_np):
    hot = 2 * NCH - 1
    sel = np.zeros((HID, 2 * 2 * NCH - 1), dtype=np.float32)
    sel[:, hot] = 1.0 / HID
    return sel.astype(act_np)


def kernel(**inputs):
    from concourse.bass_utils import run_bass_kernel_spmd

    h = np.asarray(inputs["h"], dtype=np.float32)
    coord = np.asarray(inputs["coord"], dtype=np.float32)
    msg_w1 = np.asarray(inputs["msg_w1"], dtype=np.float32)
    msg_b1 = np.asarray(inputs["msg_b1"], dtype=np.float32)
    msg_w2 = np.asarray(inputs["msg_w2"], dtype=np.float32)
    msg_b2 = np.asarray(inputs["msg_b2"], dtype=np.float32)
    upd_w1 = np.asarray(inputs["upd_w1"], dtype=np.float32)
    upd_b1 = np.asarray(inputs["upd_b1"], dtype=np.float32)
    upd_w2 = np.asarray(inputs["upd_w2"], dtype=np.float32)
    upd_b2 = np.asarray(inputs["upd_b2"], dtype=np.float32)
    ln_g = np.asarray(inputs["ln_g"], dtype=np.float32)
    ln_b = np.asarray(inputs["ln_b"], dtype=np.float32)

    np_dt = np.dtype("bfloat16") if False else None  # placeholder
    import ml_dtypes
    act_np = {"bf16": ml_dtypes.bfloat16, "fp16": np.float16, "fp32": np.float32}[DT_NAME]

    W1a = msg_w1[:HID]
    W1b = msg_w1[HID:2 * HID]
    w1c = msg_w1[2 * HID]
    bias_u = upd_b1 + msg_b2 @ upd_w1[HID:2 * HID]
    W2s = msg_w2 / (2.0 * K)

    idx = np.arange(N)
    count = (np.minimum(idx, K) + np.minimum(N - 1 - idx, K)).astype(np.float32)
    fix = (2.0 * K) / count
    fixf = fix[:K].reshape(1, K).astype(np.float32)
    fixl = fix[N - K:].reshape(1, K).astype(np.float32)

    const = {
        "W1a": np.ascontiguousarray(W1a, dtype=act_np),
        "W1b": np.ascontiguousarray(W1b, dtype=act_np),
        "w1c": np.ascontiguousarray(w1c.reshape(1, HID), dtype=act_np),
        "w1cn": np.ascontiguousarray(-w1c.reshape(1, HID), dtype=act_np),
        "W2s": np.ascontiguousarray(W2s, dtype=act_np),
        "U1a": np.ascontiguousarray(upd_w1[:HID], dtype=act_np),
        "U1b": np.ascontiguousarray(upd_w1[HID:], dtype=act_np),
        "U2": np.ascontiguousarray(upd_w2, dtype=act_np),
        "b1c": np.ascontiguousarray(msg_b1.reshape(HID, 1), dtype=np.float32),
        "buc": np.ascontiguousarray(bias_u.reshape(HID, 1), dtype=np.float32),
        "b2u": np.ascontiguousarray(upd_b2.reshape(1, HID), dtype=act_np),
        "g_row": np.ascontiguousarray(ln_g.reshape(1, HID), dtype=act_np),
        "nb_row": np.ascontiguousarray(-ln_b.reshape(1, HID), dtype=act_np),
        "ident": np.ascontiguousarray(np.eye(HID), dtype=act_np),
        "ones_col": np.full((HID, 1), 1.0 / HID, dtype=act_np),
        "fixf": fixf,
        "fixl": fixl,
        "selb": _sel_band(act_np),
    }

    in_maps = []
    for b in range(B):
        m = dict(const)
        m["hT"] = np.ascontiguousarray(h[b].T, dtype=act_np)
        m["coordR"] = np.ascontiguousarray(coord[b].reshape(1, N), dtype=act_np)
        in_maps.append(m)

    nc = _get_compiled(DT_NAME)
    res = run_bass_kernel_spmd(nc, in_maps, core_ids=list(range(B)))
    global LAST_RESULTS
    LAST_RESULTS = res
    out = np.stack([np.asarray(res.results[b]["outT"], dtype=np.float32).T
                    for b in range(B)])
    return np.ascontiguousarray(out)



# revision 15
# speedup vs baseline: 1.0867x; 1.0867x over previous
"""Trainium2 Bass kernel for the LocalGNOBlock (windowed GNN message passing).

Math restructuring (vs the naive 12x full MLP evaluations):
  msg first layer is linear over concat([h_i, h_j, dc]):
      z_d[i] = (A - C)[i] + (B + C)[i+d] + b1,  d in {+-1..+-6}
  where A = h @ W1a, B = h @ W1b, C = coord x w1c (rank-1).
  Interior chunks fold the whole message-2nd-layer + U1b product:
      u += sum_d silu(z_d) @ (W2/12 @ U1b)     (12 matmuls, PSUM accum)
  so the "agg" tensor is never materialized except at the two boundary
  chunks (count fixup).  LayerNorm stats are per-token (channel dim on
  partitions) via band-select ones matmuls packed into one PSUM bank in
  two half-batches, so normalization of the first half overlaps pass-1
  compute of the second half.

Engine budget per 512-token chunk (targets):
  ACT   silu(12T) 5.4us + silu(s2) 0.7us            -> floor ~6.1us
  DVE   z-build 3.4 + E/D casts 1.4 + x 0.7 + norm  -> ~5.5-6.9us
  PE    ~22 matmuls x 215ns (warm clock)            -> ~5us
  GPSIMD x^2 (SBUF only - no PSUM port)             -> ~1.9us
  D_B shifted copy runs as SBUF->SBUF DMA.

Sharding: batch dim B=8 -> one batch element per NeuronCore (no halo).
Host pre/post: transpose h -> [128, N] per core, transpose back after.
"""

import numpy as np

K = 6
HID = 128
N = 16384
B = 8
EPS = 1e-5
T = 512                 # token chunk (matmul + elementwise granularity)
NCH = N // T            # 32 chunks
NHALF = NCH // 2        # stats half-batch
OFF0 = 8                # D_full column of token 0 (even, for fp16 alignment)
NCOL = N + 2 * OFF0     # D_full width

# offsets ordered in 4 stride-2 groups: (even uses D_A, odd uses D_B)
NEG_EVEN = [-6, -4, -2]
NEG_ODD = [-5, -3, -1]
POS_ODD = [1, 3, 5]
POS_EVEN = [2, 4, 6]
SEG_ORDER = NEG_EVEN + NEG_ODD + POS_ODD + POS_EVEN  # 12 segments in Z

_compiled = None


def _build_bass(dt_act):
    import concourse.bacc as bacc
    import concourse.bass as bass
    import concourse.tile as tile
    from concourse import mybir

    f32 = mybir.dt.float32
    DT = dt_act

    nc = bacc.Bacc("TRN2", target_bir_lowering=False, debug=False)

    # ---- DRAM I/O ----
    hT = nc.dram_tensor("hT", [HID, N], DT, kind="ExternalInput")
    coordR = nc.dram_tensor("coordR", [1, N], DT, kind="ExternalInput")
    W1a = nc.dram_tensor("W1a", [HID, HID], DT, kind="ExternalInput")
    W1b = nc.dram_tensor("W1b", [HID, HID], DT, kind="ExternalInput")
    w1c = nc.dram_tensor("w1c", [1, HID], DT, kind="ExternalInput")      # +w1c
    w1cn = nc.dram_tensor("w1cn", [1, HID], DT, kind="ExternalInput")    # -w1c
    W2s = nc.dram_tensor("W2s", [HID, HID], DT, kind="ExternalInput")    # W2/12
    W2u = nc.dram_tensor("W2u", [HID, HID], DT, kind="ExternalInput")    # W2/12@U1b
    U1a = nc.dram_tensor("U1a", [HID, HID], DT, kind="ExternalInput")
    U1b = nc.dram_tensor("U1b", [HID, HID], DT, kind="ExternalInput")
    U2 = nc.dram_tensor("U2", [HID, HID], DT, kind="ExternalInput")
    b1c = nc.dram_tensor("b1c", [HID, 1], f32, kind="ExternalInput")      # msg_b1
    buc = nc.dram_tensor("buc", [HID, 1], f32, kind="ExternalInput")      # upd_b1+b2@U1b
    b2uc = nc.dram_tensor("b2uc", [HID, 1], f32, kind="ExternalInput")    # upd_b2 col
    lnbc = nc.dram_tensor("lnbc", [HID, 1], f32, kind="ExternalInput")    # ln_b col
    g_row = nc.dram_tensor("g_row", [1, HID], DT, kind="ExternalInput")   # ln_g
    fixf = nc.dram_tensor("fixf", [1, K], f32, kind="ExternalInput")      # 12/count head
    fixl = nc.dram_tensor("fixl", [1, K], f32, kind="ExternalInput")      # 12/count tail
    # band-select matrix: column 63 = 1/128, else 0 (stats row packing)
    selb = nc.dram_tensor("selb", [HID, 2 * 2 * NCH - 1], DT, kind="ExternalInput")
    outT = nc.dram_tensor("outT", [HID, N], f32, kind="ExternalOutput")

    Silu = mybir.ActivationFunctionType.Silu
    Sqrt = mybir.ActivationFunctionType.Sqrt
    HOT = 2 * NCH - 1   # hot column index in selb

    with tile.TileContext(nc) as tc:
        with (
            tc.tile_pool(name="singles", bufs=1) as singles,
            tc.tile_pool(name="big", bufs=1) as big,
            tc.tile_pool(name="work", bufs=3) as work,
            tc.tile_pool(name="zpool", bufs=3) as zpool,
            tc.tile_pool(name="opool", bufs=3) as opool,
            tc.tile_pool(name="stage", bufs=3) as stpool,
            tc.tile_pool(name="psDE", bufs=2, space="PSUM") as psDE,
            tc.tile_pool(name="psUX", bufs=2, space="PSUM") as psUX,
            tc.tile_pool(name="psPP", bufs=2, space="PSUM") as psPP,
            tc.tile_pool(name="psS", bufs=1, space="PSUM") as psS,
        ):
            # ---- constants into SBUF ----
            sW1a = singles.tile([HID, HID], DT)
            sW1b = singles.tile([HID, HID], DT)
            sW2s = singles.tile([HID, HID], DT)
            sW2u = singles.tile([HID, HID], DT)
            sU1a = singles.tile([HID, HID], DT)
            sU1b = singles.tile([HID, HID], DT)
            sU2 = singles.tile([HID, HID], DT)
            for sb, dr in [(sW1a, W1a), (sW1b, W1b), (sW2s, W2s), (sW2u, W2u),
                           (sU1a, U1a), (sU1b, U1b), (sU2, U2)]:
                nc.sync.dma_start(out=sb, in_=dr[:, :])
            sw1c = singles.tile([1, HID], DT)
            sw1cn = singles.tile([1, HID], DT)
            sg = singles.tile([1, HID], DT)
            for sb, dr in [(sw1c, w1c), (sw1cn, w1cn), (sg, g_row)]:
                nc.sync.dma_start(out=sb, in_=dr[:, :])
            sb1 = singles.tile([HID, 1], f32)
            sbu = singles.tile([HID, 1], f32)
            sb2u = singles.tile([HID, 1], f32)
            slnb = singles.tile([HID, 1], f32)
            for sb, dr in [(sb1, b1c), (sbu, buc), (sb2u, b2uc), (slnb, lnbc)]:
                nc.sync.dma_start(out=sb, in_=dr[:, :])
            # broadcast [1,6] -> [128,6] fix tiles
            sfixf = singles.tile([HID, K], f32)
            sfixl = singles.tile([HID, K], f32)

            def bcast_rows(dr):
                a = dr[0:1, :]
                return bass.AP(tensor=a.tensor, offset=a.offset,
                               ap=[[0, HID]] + list(a.ap[1:]))

            nc.gpsimd.dma_start(out=sfixf, in_=bcast_rows(fixf))
            nc.gpsimd.dma_start(out=sfixl, in_=bcast_rows(fixl))
            ssel = singles.tile([HID, 2 * 2 * NCH - 1], DT)
            nc.sync.dma_start(out=ssel, in_=selb[:, :])

            # ---- big persistent buffers ----
            D_A = big.tile([HID, NCOL], DT)      # token j at col OFF0 + j
            D_B = big.tile([HID, NCOL], DT)      # token j at col OFF0 + 1 + j
            x_full = big.tile([HID, N], DT)
            # zero halo columns of D so boundary silu stays finite
            nc.vector.memset(D_A[:, 0:OFF0], 0.0)
            nc.vector.memset(D_A[:, OFF0 + N:NCOL], 0.0)
            nc.vector.memset(D_B[:, 0:OFF0 + 1], 0.0)
            nc.vector.memset(D_B[:, OFF0 + 1 + N:NCOL], 0.0)

            # LN stats: one PSUM bank per half (chunks 16h..16h+15), i = c % 16:
            #   E[x]  -> row i       (DVE reads need 32-aligned partition start,
            #   E[x2] -> row 32 + i   so the two groups sit at offsets 0 and 32)
            st0_ps = psS.tile([4 * NHALF, T], f32, tag="st0")
            st1_ps = psS.tile([4 * NHALF, T], f32, tag="st1")
            sts = [st0_ps, st1_ps]

            # r|u rows for the normalize pass: row i = [r (T) | mu*r (T)]
            # (one tile per half so DVE writes start at partition 0)
            ru_sb0 = big.tile([NHALF, 2 * T], DT)
            ru_sb1 = big.tile([NHALF, 2 * T], DT)
            ru_sb = [ru_sb0, ru_sb1]
            seps = singles.tile([NHALF, 1], f32)
            nc.vector.memset(seps, float(EPS))

            hts = {}
            crd = {}
            zs = {}

            def load_chunk(c):
                ht = work.tile([HID, T], DT, tag="ht")
                nc.sync.dma_start(out=ht, in_=hT[:, c * T:(c + 1) * T])
                co = work.tile([1, T], DT, tag="co")
                nc.sync.dma_start(out=co, in_=coordR[:, c * T:(c + 1) * T])
                hts[c] = ht
                crd[c] = co

            def phase_a(c):
                # D chunk = W1b.T @ h  +  w1c x coord   (PSUM accumulate)
                d_ps = psDE.tile([HID, T], f32, tag="de")
                nc.tensor.matmul(d_ps, sW1b, hts[c], start=True, stop=False)
                nc.tensor.matmul(d_ps, sw1c, crd[c], start=False, stop=True)
                col = OFF0 + c * T
                nc.vector.tensor_copy(D_A[:, col:col + T], d_ps)
                # shifted copy for odd-offset alignment: SBUF->SBUF DMA
                nc.sync.dma_start(out=D_B[:, col + 1:col + 1 + T],
                                  in_=D_A[:, col:col + T])

            def seg_in1(tile_ap, col):
                # [128, 3, T] AP over D with outer column-stride 2
                s = tile_ap[:, col:col + T]
                return bass.AP(tensor=s.tensor, offset=s.offset,
                               ap=[s.ap[0], [2, 3], [1, T]])

            def zpart(t):
                # E chunk = W1a.T @ h - w1c x coord
                e_ps = psDE.tile([HID, T], f32, tag="de")
                nc.tensor.matmul(e_ps, sW1a, hts[t], start=True, stop=False)
                nc.tensor.matmul(e_ps, sw1cn, crd[t], start=False, stop=True)
                e_sb = work.tile([HID, T], DT, tag="esb")
                nc.vector.tensor_copy(e_sb, e_ps)

                # Z: 12 segments of E + shifted D, 4 stride-2 groups
                z = zpool.tile([HID, 12 * T], DT, tag="z")
                zv = z.rearrange("p (s t) -> p s t", t=T)
                e_b = bass.AP(tensor=e_sb.tensor, offset=e_sb.offset,
                              ap=[e_sb.ap[0], [0, 3], [1, T]])
                base = t * T
                groups = [
                    (D_A, OFF0 + base + NEG_EVEN[0]),
                    (D_B, OFF0 + 1 + base + NEG_ODD[0]),
                    (D_B, OFF0 + 1 + base + POS_ODD[0]),
                    (D_A, OFF0 + base + POS_EVEN[0]),
                ]
                for gi, (dbuf, col) in enumerate(groups):
                    nc.vector.tensor_tensor(
                        out=zv[:, 3 * gi:3 * gi + 3, :],
                        in0=e_b, in1=seg_in1(dbuf, col),
                        op=mybir.AluOpType.add)

                # silu over all 12 segments at once (bias = msg_b1)
                nc.scalar.activation(z, z, Silu, bias=sb1, scale=1.0)
                zs[t] = z

            def upart(t):
                ht = hts[t]
                z = zs.pop(t)
                zv = z.rearrange("p (s t) -> p s t", t=T)
                boundary = t == 0 or t == NCH - 1

                # zero invalid boundary columns (torn edges of the sequence)
                if t == 0:
                    for s, d in enumerate(SEG_ORDER):
                        if d < 0:
                            nc.vector.memset(zv[:, s, 0:-d], 0.0)
                if t == NCH - 1:
                    for s, d in enumerate(SEG_ORDER):
                        if d > 0:
                            nc.vector.memset(zv[:, s, T - d:T], 0.0)

                if boundary:
                    # old path: explicit agg + count fixup + U1b matmul
                    a_ps = psUX.tile([HID, T], f32, tag="ux")
                    for s in range(12):
                        nc.tensor.matmul(a_ps, sW2s, zv[:, s, :],
                                         start=(s == 0), stop=(s == 11))
                    agg = work.tile([HID, T], DT, tag="agg_sb")
                    nc.vector.tensor_copy(agg, a_ps)
                    if t == 0:
                        nc.vector.tensor_tensor(
                            out=agg[:, 0:K], in0=a_ps[:, 0:K],
                            in1=sfixf, op=mybir.AluOpType.mult)
                    else:
                        nc.vector.tensor_tensor(
                            out=agg[:, T - K:T], in0=a_ps[:, T - K:T],
                            in1=sfixl, op=mybir.AluOpType.mult)
                    u_ps = psUX.tile([HID, T], f32, tag="ux")
                    nc.tensor.matmul(u_ps, sU1a, ht, start=True, stop=False)
                    nc.tensor.matmul(u_ps, sU1b, agg, start=False, stop=True)
                else:
                    # folded: u = U1a.T@h + sum_s (W2u).T @ silu_s
                    u_ps = psUX.tile([HID, T], f32, tag="ux")
                    nc.tensor.matmul(u_ps, sU1a, ht, start=True, stop=False)
                    for s in range(12):
                        nc.tensor.matmul(u_ps, sW2u, zv[:, s, :],
                                         start=False, stop=(s == 11))
                s2 = work.tile([HID, T], DT, tag="s2")
                nc.scalar.activation(s2, u_ps, Silu, bias=sbu, scale=1.0)

                # x = (U2.T@s2 + b2u) + h   (single fused DVE op)
                x_ps = psUX.tile([HID, T], f32, tag="ux")
                nc.tensor.matmul(x_ps, sU2, s2, start=True, stop=True)
                base = t * T
                x_sb = x_full[:, base:base + T]
                nc.vector.scalar_tensor_tensor(
                    out=x_sb, in0=x_ps, scalar=sb2u, in1=ht,
                    op0=mybir.AluOpType.add, op1=mybir.AluOpType.add)
                x2 = work.tile([HID, T], DT, tag="x2")
                nc.gpsimd.tensor_tensor(out=x2, in0=x_sb, in1=x_sb,
                                        op=mybir.AluOpType.mult)
                # stats rows in the half's own bank: E[x] row i, E[x2] row 32+i
                h_, i_ = t // NHALF, t % NHALF
                st = sts[h_]
                r_e2 = 2 * NHALF + i_
                first = i_ == 0
                last = i_ == NHALF - 1
                nc.tensor.matmul(st[:, :], ssel[:, HOT - i_:HOT - i_ + 4 * NHALF],
                                 x_sb, start=first, stop=False)
                nc.tensor.matmul(st[:, :], ssel[:, HOT - r_e2:HOT - r_e2 + 4 * NHALF],
                                 x2, start=False, stop=last)

            def stats_math(h_):
                # batched per-token LN stats for chunks 16h..16h+15
                ru = ru_sb[h_]
                # E[x] rows 0:16 to SBUF; E[x2] stays in PSUM (rows 32:48 —
                # 32-aligned; PSUM+SB operand bases may differ, SB+SB may not)
                ex_sb = work.tile([NHALF, T], f32, tag="ex")
                nc.vector.tensor_copy(ex_sb, sts[h_][0:NHALF, :])
                t1 = work.tile([NHALF, T], f32, tag="t1")
                nc.vector.tensor_tensor(out=t1, in0=ex_sb, in1=ex_sb,
                                        op=mybir.AluOpType.mult)
                var = work.tile([NHALF, T], f32, tag="var")
                nc.vector.tensor_tensor(
                    out=var, in0=sts[h_][2 * NHALF:3 * NHALF, :], in1=t1,
                    op=mybir.AluOpType.subtract)
                nc.scalar.activation(var, var, Sqrt, bias=seps, scale=1.0)
                with nc.allow_low_precision(reason="rstd rows feed fp16 matmuls"):
                    nc.vector.reciprocal(out=ru[:, 0:T], in_=var)
                nc.vector.tensor_tensor(out=ru[:, T:2 * T], in0=ex_sb,
                                        in1=ru[:, 0:T],
                                        op=mybir.AluOpType.mult)

            def pass2(t):
                # normalize chunk t:  out = x*(g x r) + lnb - (g x mu*r)
                base = t * T
                ru = stpool.tile([1, 2 * T], DT, tag="ru")
                src = ru_sb[t // NHALF]
                nc.sync.dma_start(out=ru, in_=src[t % NHALF:t % NHALF + 1, :])
                p1 = psPP.tile([HID, T], f32, tag="pp")
                nc.tensor.matmul(p1, sg, ru[0:1, 0:T], start=True, stop=True)
                p2 = psPP.tile([HID, T], f32, tag="pp")
                nc.tensor.matmul(p2, sg, ru[0:1, T:2 * T], start=True, stop=True)
                o = opool.tile([HID, T], f32, tag="o")
                nc.vector.tensor_tensor(out=o, in0=x_full[:, base:base + T],
                                        in1=p1, op=mybir.AluOpType.mult)
                nc.vector.scalar_tensor_tensor(
                    out=o, in0=o, scalar=slnb, in1=p2,
                    op0=mybir.AluOpType.add, op1=mybir.AluOpType.subtract)
                nc.sync.dma_start(out=outT[:, base:base + T], in_=o)

            # ---------------- fused pipeline ----------------
            # iter c: load(c+1); phase_a(c); zpart(c-1); upart(c-2)
            # ACT order: ... silu_z(c-1) then silu_s2(c-2) ... keeps ACT dense.
            p2q = []
            load_chunk(0)
            for c in range(NCH + 2):
                if c < NCH:
                    if c + 1 < NCH:
                        load_chunk(c + 1)
                    phase_a(c)
                if 1 <= c <= NCH:
                    zpart(c - 1)
                if c >= 2:
                    upart(c - 2)
                    if c - 2 == NHALF - 1:
                        stats_math(0)
                        p2q.extend(range(NHALF))
                # drain one queued normalize chunk per iteration
                if p2q and c >= NHALF + 2:
                    pass2(p2q.pop(0))
            stats_math(1)
            for t in range(NHALF, NCH):
                p2q.append(t)
            while p2q:
                pass2(p2q.pop(0))

    nc.compile()
    return nc


def _get_compiled(dt_name):
    global _compiled
    if _compiled is None:
        from concourse import mybir
        dt = {"bf16": mybir.dt.bfloat16, "fp16": mybir.dt.float16,
              "fp32": mybir.dt.float32}[dt_name]
        _compiled = _build_bass(dt)
    return _compiled


DT_NAME = "fp16"


def _sel_band(act_np):
    sel = np.zeros((HID, 2 * 2 * NCH - 1), dtype=np.float32)
    sel[:, 2 * NCH - 1] = 1.0 / HID
    return sel.astype(act_np)


def kernel(**inputs):
    from concourse.bass_utils import run_bass_kernel_spmd

    h = np.asarray(inputs["h"], dtype=np.float32)
    coord = np.asarray(inputs["coord"], dtype=np.float32)
    msg_w1 = np.asarray(inputs["msg_w1"], dtype=np.float32)
    msg_b1 = np.asarray(inputs["msg_b1"], dtype=np.float32)
    msg_w2 = np.asarray(inputs["msg_w2"], dtype=np.float32)
    msg_b2 = np.asarray(inputs["msg_b2"], dtype=np.float32)
    upd_w1 = np.asarray(inputs["upd_w1"], dtype=np.float32)
    upd_b1 = np.asarray(inputs["upd_b1"], dtype=np.float32)
    upd_w2 = np.asarray(inputs["upd_w2"], dtype=np.float32)
    upd_b2 = np.asarray(inputs["upd_b2"], dtype=np.float32)
    ln_g = np.asarray(inputs["ln_g"], dtype=np.float32)
    ln_b = np.asarray(inputs["ln_b"], dtype=np.float32)

    import ml_dtypes
    act_np = {"bf16": ml_dtypes.bfloat16, "fp16": np.float16,
              "fp32": np.float32}[DT_NAME]

    W1a = msg_w1[:HID]
    W1b = msg_w1[HID:2 * HID]
    w1c = msg_w1[2 * HID]
    U1b_f = upd_w1[HID:2 * HID]
    bias_u = upd_b1 + msg_b2 @ U1b_f
    W2s = msg_w2 / (2.0 * K)
    W2u = W2s @ U1b_f

    idx = np.arange(N)
    count = (np.minimum(idx, K) + np.minimum(N - 1 - idx, K)).astype(np.float32)
    fix = (2.0 * K) / count
    fixf = fix[:K].reshape(1, K).astype(np.float32)
    fixl = fix[N - K:].reshape(1, K).astype(np.float32)

    const = {
        "W1a": np.ascontiguousarray(W1a, dtype=act_np),
        "W1b": np.ascontiguousarray(W1b, dtype=act_np),
        "w1c": np.ascontiguousarray(w1c.reshape(1, HID), dtype=act_np),
        "w1cn": np.ascontiguousarray(-w1c.reshape(1, HID), dtype=act_np),
        "W2s": np.ascontiguousarray(W2s, dtype=act_np),
        "W2u": np.ascontiguousarray(W2u, dtype=act_np),
        "U1a": np.ascontiguousarray(upd_w1[:HID], dtype=act_np),
        "U1b": np.ascontiguousarray(U1b_f, dtype=act_np),
        "U2": np.ascontiguousarray(upd_w2, dtype=act_np),
        "b1c": np.ascontiguousarray(msg_b1.reshape(HID, 1), dtype=np.float32),
        "buc": np.ascontiguousarray(bias_u.reshape(HID, 1), dtype=np.float32),
        "b2uc": np.ascontiguousarray(upd_b2.reshape(HID, 1), dtype=np.float32),
        "lnbc": np.ascontiguousarray(ln_b.reshape(HID, 1), dtype=np.float32),
        "g_row": np.ascontiguousarray(ln_g.reshape(1, HID), dtype=act_np),
        "fixf": fixf,
        "fixl": fixl,
        "selb": _sel_band(act_np),
    }

    in_maps = []
    for b in range(B):
        m = dict(const)
        m["hT"] = np.ascontiguousarray(h[b].T, dtype=act_np)
        m["coordR"] = np.ascontiguousarray(coord[b].reshape(1, N), dtype=act_np)
        in_maps.append(m)

    nc = _get_compiled(DT_NAME)
    res = run_bass_kernel_spmd(nc, in_maps, core_ids=list(range(B)))
    global LAST_RESULTS
    LAST_RESULTS = res
    out = np.stack([np.asarray(res.results[b]["outT"], dtype=np.float32).T
                    for b in range(B)])
    return np.ascontiguousarray(out)


# revision 18
# speedup vs baseline: 1.1914x; 1.0963x over previous
"""Trainium2 Bass kernel for the LocalGNOBlock (windowed GNN message passing).

Math restructuring (vs the naive 12x full MLP evaluations):
  msg first layer is linear over concat([h_i, h_j, dc]):
      z_d[i] = (A - C)[i] + (B + C)[i+d] + b1,  d in {+-1..+-6}
  where A = h @ W1a, B = h @ W1b, C = coord x w1c (rank-1).
  Interior chunks fold the whole message-2nd-layer + U1b product:
      u += sum_d silu(z_d) @ (W2/12 @ U1b)     (12 matmuls, PSUM accum)
  so the "agg" tensor is never materialized except at the two boundary
  chunks (count fixup).  LayerNorm stats are per-token (channel dim on
  partitions) via band-select ones matmuls packed into one PSUM bank in
  two half-batches, so normalization of the first half overlaps pass-1
  compute of the second half.

Engine budget per 512-token chunk (targets):
  ACT   silu(12T) 5.4us + silu(s2) 0.7us            -> floor ~6.1us
  DVE   z-build 3.4 + E/D casts 1.4 + x 0.7 + norm  -> ~5.5-6.9us
  PE    ~22 matmuls x 215ns (warm clock)            -> ~5us
  GPSIMD x^2 (SBUF only - no PSUM port)             -> ~1.9us
  D_B shifted copy runs as SBUF->SBUF DMA.

Sharding: batch dim B=8 -> one batch element per NeuronCore (no halo).
Host pre/post: transpose h -> [128, N] per core, transpose back after.
"""

import numpy as np

K = 6
HID = 128
N = 16384
B = 8
EPS = 1e-5
T = 512                 # token chunk (matmul + elementwise granularity)
NCH = N // T            # 32 chunks
NHALF = NCH // 2        # stats half-batch
OFF0 = 8                # D_full column of token 0 (even, for fp16 alignment)
NCOL = N + 2 * OFF0     # D_full width

# offsets ordered in 4 stride-2 groups: (even uses D_A, odd uses D_B)
NEG_EVEN = [-6, -4, -2]
NEG_ODD = [-5, -3, -1]
POS_ODD = [1, 3, 5]
POS_EVEN = [2, 4, 6]
SEG_ORDER = NEG_EVEN + NEG_ODD + POS_ODD + POS_EVEN  # 12 segments in Z

_compiled = None


def _build_bass(dt_act):
    import concourse.bacc as bacc
    import concourse.bass as bass
    import concourse.tile as tile
    from concourse import mybir

    f32 = mybir.dt.float32
    DT = dt_act

    nc = bacc.Bacc("TRN2", target_bir_lowering=False, debug=False)

    # ---- DRAM I/O ----
    hT = nc.dram_tensor("hT", [HID, N], DT, kind="ExternalInput")
    coordR = nc.dram_tensor("coordR", [1, N], DT, kind="ExternalInput")
    W1a = nc.dram_tensor("W1a", [HID, HID], DT, kind="ExternalInput")
    W1b = nc.dram_tensor("W1b", [HID, HID], DT, kind="ExternalInput")
    w1c = nc.dram_tensor("w1c", [1, HID], DT, kind="ExternalInput")      # +w1c
    w1cn = nc.dram_tensor("w1cn", [1, HID], DT, kind="ExternalInput")    # -w1c
    W2s = nc.dram_tensor("W2s", [HID, HID], DT, kind="ExternalInput")    # W2/12
    W2u = nc.dram_tensor("W2u", [HID, HID], DT, kind="ExternalInput")    # W2/12@U1b
    U1a = nc.dram_tensor("U1a", [HID, HID], DT, kind="ExternalInput")
    U1b = nc.dram_tensor("U1b", [HID, HID], DT, kind="ExternalInput")
    U2 = nc.dram_tensor("U2", [HID, HID], DT, kind="ExternalInput")
    b1c = nc.dram_tensor("b1c", [HID, 1], f32, kind="ExternalInput")      # msg_b1
    buc = nc.dram_tensor("buc", [HID, 1], f32, kind="ExternalInput")      # upd_b1+b2@U1b
    b2uc = nc.dram_tensor("b2uc", [HID, 1], f32, kind="ExternalInput")    # upd_b2 col
    lnbc = nc.dram_tensor("lnbc", [HID, 1], f32, kind="ExternalInput")    # ln_b col
    g_row = nc.dram_tensor("g_row", [1, HID], DT, kind="ExternalInput")   # ln_g
    fixf = nc.dram_tensor("fixf", [1, K], f32, kind="ExternalInput")      # 12/count head
    fixl = nc.dram_tensor("fixl", [1, K], f32, kind="ExternalInput")      # 12/count tail
    # band-select matrix: column 63 = 1/128, else 0 (stats row packing)
    selb = nc.dram_tensor("selb", [HID, 2 * 2 * NCH - 1], DT, kind="ExternalInput")
    outT = nc.dram_tensor("outT", [HID, N], f32, kind="ExternalOutput")

    Silu = mybir.ActivationFunctionType.Silu
    Sqrt = mybir.ActivationFunctionType.Sqrt
    HOT = 2 * NCH - 1   # hot column index in selb

    with tile.TileContext(nc) as tc:
        with (
            tc.tile_pool(name="singles", bufs=1) as singles,
            tc.tile_pool(name="big", bufs=1) as big,
            tc.tile_pool(name="work", bufs=3) as work,
            tc.tile_pool(name="zpool", bufs=3) as zpool,
            tc.tile_pool(name="opool", bufs=3) as opool,
            tc.tile_pool(name="stage", bufs=3) as stpool,
            tc.tile_pool(name="psDE", bufs=2, space="PSUM") as psDE,
            tc.tile_pool(name="psUX", bufs=3, space="PSUM") as psUX,
            tc.tile_pool(name="psPP", bufs=1, space="PSUM") as psPP,
            tc.tile_pool(name="psS", bufs=1, space="PSUM") as psS,
        ):
            # ---- constants into SBUF ----
            sW1a = singles.tile([HID, HID], DT)
            sW1b = singles.tile([HID, HID], DT)
            sW2s = singles.tile([HID, HID], DT)
            sW2u = singles.tile([HID, HID], DT)
            sU1a = singles.tile([HID, HID], DT)
            sU1b = singles.tile([HID, HID], DT)
            sU2 = singles.tile([HID, HID], DT)
            for sb, dr in [(sW1a, W1a), (sW1b, W1b), (sW2s, W2s), (sW2u, W2u),
                           (sU1a, U1a), (sU1b, U1b), (sU2, U2)]:
                nc.sync.dma_start(out=sb, in_=dr[:, :])
            sw1c = singles.tile([1, HID], DT)
            sw1cn = singles.tile([1, HID], DT)
            sg = singles.tile([1, HID], DT)
            for sb, dr in [(sw1c, w1c), (sw1cn, w1cn), (sg, g_row)]:
                nc.sync.dma_start(out=sb, in_=dr[:, :])
            sb1 = singles.tile([HID, 1], f32)
            sbu = singles.tile([HID, 1], f32)
            sb2u = singles.tile([HID, 1], f32)
            slnb = singles.tile([HID, 1], f32)
            for sb, dr in [(sb1, b1c), (sbu, buc), (sb2u, b2uc), (slnb, lnbc)]:
                nc.sync.dma_start(out=sb, in_=dr[:, :])
            # broadcast [1,6] -> [128,6] fix tiles
            sfixf = singles.tile([HID, K], f32)
            sfixl = singles.tile([HID, K], f32)

            def bcast_rows(dr):
                a = dr[0:1, :]
                return bass.AP(tensor=a.tensor, offset=a.offset,
                               ap=[[0, HID]] + list(a.ap[1:]))

            nc.gpsimd.dma_start(out=sfixf, in_=bcast_rows(fixf))
            nc.gpsimd.dma_start(out=sfixl, in_=bcast_rows(fixl))
            ssel = singles.tile([HID, 2 * 2 * NCH - 1], DT)
            nc.sync.dma_start(out=ssel, in_=selb[:, :])

            # ---- big persistent buffers ----
            D_A = big.tile([HID, NCOL], DT)      # token j at col OFF0 + j
            D_B = big.tile([HID, NCOL], DT)      # token j at col OFF0 + 1 + j
            x_full = big.tile([HID, N], DT)
            # zero halo columns of D so boundary silu stays finite
            nc.vector.memset(D_A[:, 0:OFF0], 0.0)
            nc.vector.memset(D_A[:, OFF0 + N:NCOL], 0.0)
            nc.vector.memset(D_B[:, 0:OFF0 + 1], 0.0)
            nc.vector.memset(D_B[:, OFF0 + 1 + N:NCOL], 0.0)

            # LN stats: one PSUM bank per half (chunks 16h..16h+15), i = c % 16:
            #   E[x]  -> row i       (DVE reads need 32-aligned partition start,
            #   E[x2] -> row 32 + i   so the two groups sit at offsets 0 and 32)
            st0_ps = psS.tile([4 * NHALF, T], f32, tag="st0")
            st1_ps = psS.tile([4 * NHALF, T], f32, tag="st1")
            sts = [st0_ps, st1_ps]

            # r|u rows for the normalize pass: row i = [r (T) | mu*r (T)]
            # (one tile per half so DVE writes start at partition 0)
            ru_sb0 = big.tile([NHALF, 2 * T], DT)
            ru_sb1 = big.tile([NHALF, 2 * T], DT)
            ru_sb = [ru_sb0, ru_sb1]
            seps = singles.tile([NHALF, 1], f32)
            nc.vector.memset(seps, float(EPS))

            hts = {}
            crd = {}
            zs = {}

            def load_chunk(c):
                ht = work.tile([HID, T], DT, tag="ht")
                nc.sync.dma_start(out=ht, in_=hT[:, c * T:(c + 1) * T])
                co = work.tile([1, T], DT, tag="co")
                nc.sync.dma_start(out=co, in_=coordR[:, c * T:(c + 1) * T])
                hts[c] = ht
                crd[c] = co

            def phase_a(c):
                # D chunk = W1b.T @ h  +  w1c x coord   (PSUM accumulate)
                d_ps = psDE.tile([HID, T], f32, tag="de")
                nc.tensor.matmul(d_ps, sW1b, hts[c], start=True, stop=False)
                nc.tensor.matmul(d_ps, sw1c, crd[c], start=False, stop=True)
                col = OFF0 + c * T
                nc.vector.tensor_copy(D_A[:, col:col + T], d_ps)
                # shifted copy for odd-offset alignment: SBUF->SBUF DMA
                nc.sync.dma_start(out=D_B[:, col + 1:col + 1 + T],
                                  in_=D_A[:, col:col + T])

            def seg_in1(tile_ap, col, n):
                # [128, n, T] AP over D with outer column-stride 2
                s = tile_ap[:, col:col + T]
                return bass.AP(tensor=s.tensor, offset=s.offset,
                               ap=[s.ap[0], [2, n], [1, T]])

            def e_bcast(e_sb, n):
                return bass.AP(tensor=e_sb.tensor, offset=e_sb.offset,
                               ap=[e_sb.ap[0], [0, n], [1, T]])

            esbs = {}
            aps = {}
            us = {}
            s2s = {}

            def zpartA(t):
                # E chunk = W1a.T @ h - w1c x coord
                e_ps = psDE.tile([HID, T], f32, tag="de")
                nc.tensor.matmul(e_ps, sW1a, hts[t], start=True, stop=False)
                nc.tensor.matmul(e_ps, sw1cn, crd[t], start=False, stop=True)
                e_sb = work.tile([HID, T], DT, tag="esb")
                nc.vector.tensor_copy(e_sb, e_ps)
                esbs[t] = e_sb

                z = zpool.tile([HID, 12 * T], DT, tag="z")
                zv = z.rearrange("p (s t) -> p s t", t=T)
                base = t * T
                # seg 0 (d=-6) on gpsimd; segs 1-2 / 3-5 on DVE
                nc.gpsimd.tensor_tensor(
                    out=zv[:, 0, :], in0=e_sb,
                    in1=D_A[:, OFF0 + base - 6:OFF0 + base - 6 + T],
                    op=mybir.AluOpType.add)
                nc.vector.tensor_tensor(
                    out=zv[:, 1:3, :], in0=e_bcast(e_sb, 2),
                    in1=seg_in1(D_A, OFF0 + base - 4, 2), op=mybir.AluOpType.add)
                nc.vector.tensor_tensor(
                    out=zv[:, 3:6, :], in0=e_bcast(e_sb, 3),
                    in1=seg_in1(D_B, OFF0 + 1 + base - 5, 3),
                    op=mybir.AluOpType.add)
                # silu first half (segs 0-5), bias = msg_b1
                nc.scalar.activation(z[:, 0:6 * T], z[:, 0:6 * T], Silu,
                                     bias=sb1, scale=1.0)
                zs[t] = (z, zv)

            def zpartB(t):
                z, zv = zs[t]
                e_sb = esbs.pop(t)
                base = t * T
                nc.vector.tensor_tensor(
                    out=zv[:, 6:9, :], in0=e_bcast(e_sb, 3),
                    in1=seg_in1(D_B, OFF0 + 1 + base + 1, 3),
                    op=mybir.AluOpType.add)
                nc.vector.tensor_tensor(
                    out=zv[:, 9:12, :], in0=e_bcast(e_sb, 3),
                    in1=seg_in1(D_A, OFF0 + base + 2, 3), op=mybir.AluOpType.add)
                # silu second half (segs 6-11)
                nc.scalar.activation(z[:, 6 * T:12 * T], z[:, 6 * T:12 * T],
                                     Silu, bias=sb1, scale=1.0)

            def msgA(t):
                # first 6 message matmuls (needs silu half 0)
                _, zv = zs[t]
                boundary = t == 0 or t == NCH - 1
                if t == 0:
                    for s, d in enumerate(SEG_ORDER):
                        if d < 0:
                            nc.vector.memset(zv[:, s, 0:-d], 0.0)
                if boundary:
                    a_ps = psUX.tile([HID, T], f32, tag="ux")
                    for s in range(6):
                        nc.tensor.matmul(a_ps, sW2s, zv[:, s, :],
                                         start=(s == 0), stop=False)
                    aps[t] = a_ps
                else:
                    u_ps = psUX.tile([HID, T], f32, tag="ux")
                    nc.tensor.matmul(u_ps, sU1a, hts[t], start=True, stop=False)
                    for s in range(6):
                        nc.tensor.matmul(u_ps, sW2u, zv[:, s, :],
                                         start=False, stop=False)
                    us[t] = u_ps

            def msgB(t):
                # last 6 message matmuls (needs silu half 1)
                _, zv = zs.pop(t)
                boundary = t == 0 or t == NCH - 1
                if t == NCH - 1:
                    for s, d in enumerate(SEG_ORDER):
                        if d > 0:
                            nc.vector.memset(zv[:, s, T - d:T], 0.0)
                tgt = aps[t] if boundary else us[t]
                w = sW2s if boundary else sW2u
                for s in range(6, 12):
                    nc.tensor.matmul(tgt, w, zv[:, s, :],
                                     start=False, stop=(s == 11))

            def s2em(t):
                # interior only: silu of update-MLP hidden (between silu halves)
                s2 = work.tile([HID, T], DT, tag="s2")
                nc.scalar.activation(s2, us.pop(t), Silu, bias=sbu, scale=1.0)
                s2s[t] = s2

            def tail(t):
                ht = hts[t]
                boundary = t == 0 or t == NCH - 1
                if boundary:
                    a_ps = aps.pop(t)
                    agg = work.tile([HID, T], DT, tag="agg_sb")
                    nc.vector.tensor_copy(agg, a_ps)
                    if t == 0:
                        nc.vector.tensor_tensor(
                            out=agg[:, 0:K], in0=a_ps[:, 0:K],
                            in1=sfixf, op=mybir.AluOpType.mult)
                    else:
                        nc.vector.tensor_tensor(
                            out=agg[:, T - K:T], in0=a_ps[:, T - K:T],
                            in1=sfixl, op=mybir.AluOpType.mult)
                    u_ps = psUX.tile([HID, T], f32, tag="ux")
                    nc.tensor.matmul(u_ps, sU1a, ht, start=True, stop=False)
                    nc.tensor.matmul(u_ps, sU1b, agg, start=False, stop=True)
                    s2 = work.tile([HID, T], DT, tag="s2")
                    nc.scalar.activation(s2, u_ps, Silu, bias=sbu, scale=1.0)
                else:
                    s2 = s2s.pop(t)

                # x = (U2.T@s2 + b2u) + h   (single fused DVE op)
                x_ps = psUX.tile([HID, T], f32, tag="ux")
                nc.tensor.matmul(x_ps, sU2, s2, start=True, stop=True)
                base = t * T
                x_sb = x_full[:, base:base + T]
                nc.vector.scalar_tensor_tensor(
                    out=x_sb, in0=x_ps, scalar=sb2u, in1=ht,
                    op0=mybir.AluOpType.add, op1=mybir.AluOpType.add)
                x2 = work.tile([HID, T], DT, tag="x2")
                nc.gpsimd.tensor_tensor(out=x2, in0=x_sb, in1=x_sb,
                                        op=mybir.AluOpType.mult)
                # stats rows in the half's own bank: E[x] row i, E[x2] row 32+i
                h_, i_ = t // NHALF, t % NHALF
                st = sts[h_]
                r_e2 = 2 * NHALF + i_
                first = i_ == 0
                last = i_ == NHALF - 1
                nc.tensor.matmul(st[:, :], ssel[:, HOT - i_:HOT - i_ + 4 * NHALF],
                                 x_sb, start=first, stop=False)
                nc.tensor.matmul(st[:, :], ssel[:, HOT - r_e2:HOT - r_e2 + 4 * NHALF],
                                 x2, start=False, stop=last)

            def stats_math(h_):
                # batched per-token LN stats for chunks 16h..16h+15
                ru = ru_sb[h_]
                # E[x] rows 0:16 to SBUF; E[x2] stays in PSUM (rows 32:48 —
                # 32-aligned; PSUM+SB operand bases may differ, SB+SB may not)
                ex_sb = work.tile([NHALF, T], f32, tag="ex")
                nc.vector.tensor_copy(ex_sb, sts[h_][0:NHALF, :])
                t1 = work.tile([NHALF, T], f32, tag="t1")
                nc.vector.tensor_tensor(out=t1, in0=ex_sb, in1=ex_sb,
                                        op=mybir.AluOpType.mult)
                var = work.tile([NHALF, T], f32, tag="var")
                nc.vector.tensor_tensor(
                    out=var, in0=sts[h_][2 * NHALF:3 * NHALF, :], in1=t1,
                    op=mybir.AluOpType.subtract)
                nc.scalar.activation(var, var, Sqrt, bias=seps, scale=1.0)
                with nc.allow_low_precision(reason="rstd rows feed fp16 matmuls"):
                    nc.vector.reciprocal(out=ru[:, 0:T], in_=var)
                nc.vector.tensor_tensor(out=ru[:, T:2 * T], in0=ex_sb,
                                        in1=ru[:, 0:T],
                                        op=mybir.AluOpType.mult)

            def pass2(t):
                # normalize chunk t:  out = x*(g x r) + lnb - (g x mu*r)
                base = t * T
                ru = stpool.tile([1, 2 * T], DT, tag="ru")
                src = ru_sb[t // NHALF]
                nc.sync.dma_start(out=ru, in_=src[t % NHALF:t % NHALF + 1, :])
                p1 = psPP.tile([HID, T], f32, tag="pp")
                nc.tensor.matmul(p1, sg, ru[0:1, 0:T], start=True, stop=True)
                p2 = psPP.tile([HID, T], f32, tag="pp")
                nc.tensor.matmul(p2, sg, ru[0:1, T:2 * T], start=True, stop=True)
                o = opool.tile([HID, T], f32, tag="o")
                nc.vector.tensor_tensor(out=o, in0=x_full[:, base:base + T],
                                        in1=p1, op=mybir.AluOpType.mult)
                nc.vector.scalar_tensor_tensor(
                    out=o, in0=o, scalar=slnb, in1=p2,
                    op0=mybir.AluOpType.add, op1=mybir.AluOpType.subtract)
                nc.sync.dma_start(out=outT[:, base:base + T], in_=o)

            # ---------------- fused pipeline ----------------
            # iter c emits (interleaved so ACT runs silu_h0(c-1), s2(c-2),
            # silu_h1(c-1) back-to-back and the PE is never starved):
            p2q = []
            load_chunk(0)
            for c in range(NCH + 2):
                if c < NCH:
                    if c + 1 < NCH:
                        load_chunk(c + 1)
                    phase_a(c)
                if 1 <= c <= NCH:
                    zpartA(c - 1)
                if 2 <= c <= NCH + 1:
                    msgB(c - 2)
                    if 0 < c - 2 < NCH - 1:
                        s2em(c - 2)
                if 1 <= c <= NCH:
                    zpartB(c - 1)
                    msgA(c - 1)
                if 2 <= c <= NCH + 1:
                    tail(c - 2)
                    if c - 2 == NHALF - 1:
                        stats_math(0)
                        p2q.extend(range(NHALF))
                # drain one queued normalize chunk per iteration
                if p2q and c >= NHALF + 2:
                    pass2(p2q.pop(0))
            stats_math(1)
            for t in range(NHALF, NCH):
                p2q.append(t)
            while p2q:
                pass2(p2q.pop(0))

    nc.compile()
    return nc


def _get_compiled(dt_name):
    global _compiled
    if _compiled is None:
        from concourse import mybir
        dt = {"bf16": mybir.dt.bfloat16, "fp16": mybir.dt.float16,
              "fp32": mybir.dt.float32}[dt_name]
        _compiled = _build_bass(dt)
    return _compiled


DT_NAME = "fp16"


def _sel_band(act_np):
    sel = np.zeros((HID, 2 * 2 * NCH - 1), dtype=np.float32)
    sel[:, 2 * NCH - 1] = 1.0 / HID
    return sel.astype(act_np)


def kernel(**inputs):
    from concourse.bass_utils import run_bass_kernel_spmd

    h = np.asarray(inputs["h"], dtype=np.float32)
    coord = np.asarray(inputs["coord"], dtype=np.float32)
    msg_w1 = np.asarray(inputs["msg_w1"], dtype=np.float32)
    msg_b1 = np.asarray(inputs["msg_b1"], dtype=np.float32)
    msg_w2 = np.asarray(inputs["msg_w2"], dtype=np.float32)
    msg_b2 = np.asarray(inputs["msg_b2"], dtype=np.float32)
    upd_w1 = np.asarray(inputs["upd_w1"], dtype=np.float32)
    upd_b1 = np.asarray(inputs["upd_b1"], dtype=np.float32)
    upd_w2 = np.asarray(inputs["upd_w2"], dtype=np.float32)
    upd_b2 = np.asarray(inputs["upd_b2"], dtype=np.float32)
    ln_g = np.asarray(inputs["ln_g"], dtype=np.float32)
    ln_b = np.asarray(inputs["ln_b"], dtype=np.float32)

    import ml_dtypes
    act_np = {"bf16": ml_dtypes.bfloat16, "fp16": np.float16,
              "fp32": np.float32}[DT_NAME]

    W1a = msg_w1[:HID]
    W1b = msg_w1[HID:2 * HID]
    w1c = msg_w1[2 * HID]
    U1b_f = upd_w1[HID:2 * HID]
    bias_u = upd_b1 + msg_b2 @ U1b_f
    W2s = msg_w2 / (2.0 * K)
    W2u = W2s @ U1b_f

    idx = np.arange(N)
    count = (np.minimum(idx, K) + np.minimum(N - 1 - idx, K)).astype(np.float32)
    fix = (2.0 * K) / count
    fixf = fix[:K].reshape(1, K).astype(np.float32)
    fixl = fix[N - K:].reshape(1, K).astype(np.float32)

    const = {
        "W1a": np.ascontiguousarray(W1a, dtype=act_np),
        "W1b": np.ascontiguousarray(W1b, dtype=act_np),
        "w1c": np.ascontiguousarray(w1c.reshape(1, HID), dtype=act_np),
        "w1cn": np.ascontiguousarray(-w1c.reshape(1, HID), dtype=act_np),
        "W2s": np.ascontiguousarray(W2s, dtype=act_np),
        "W2u": np.ascontiguousarray(W2u, dtype=act_np),
        "U1a": np.ascontiguousarray(upd_w1[:HID], dtype=act_np),
        "U1b": np.ascontiguousarray(U1b_f, dtype=act_np),
        "U2": np.ascontiguousarray(upd_w2, dtype=act_np),
        "b1c": np.ascontiguousarray(msg_b1.reshape(HID, 1), dtype=np.float32),
        "buc": np.ascontiguousarray(bias_u.reshape(HID, 1), dtype=np.float32),
        "b2uc": np.ascontiguousarray(upd_b2.reshape(HID, 1), dtype=np.float32),
        "lnbc": np.ascontiguousarray(ln_b.reshape(HID, 1), dtype=np.float32),
        "g_row": np.ascontiguousarray(ln_g.reshape(1, HID), dtype=act_np),
        "fixf": fixf,
        "fixl": fixl,
        "selb": _sel_band(act_np),
    }

    in_maps = []
    for b in range(B):
        m = dict(const)
        m["hT"] = np.ascontiguousarray(h[b].T, dtype=act_np)
        m["coordR"] = np.ascontiguousarray(coord[b].reshape(1, N), dtype=act_np)
        in_maps.append(m)

    nc = _get_compiled(DT_NAME)
    res = run_bass_kernel_spmd(nc, in_maps, core_ids=list(range(B)))
    global LAST_RESULTS
    LAST_RESULTS = res
    out = np.stack([np.asarray(res.results[b]["outT"], dtype=np.float32).T
                    for b in range(B)])
    return np.ascontiguousarray(out)


# revision 24
# speedup vs baseline: 1.3501x; 1.1332x over previous
"""Trainium2 Bass kernel for the LocalGNOBlock (windowed GNN message passing).

Math restructuring (vs the naive 12x full MLP evaluations):
  msg first layer is linear over concat([h_i, h_j, dc]):
      z_d[i] = (A - C)[i] + (B + C)[i+d] + b1,  d in {+-1..+-6}
  where A = h @ W1a, B = h @ W1b, C = coord x w1c (rank-1).
  Interior chunks fold the whole message-2nd-layer + U1b product:
      u += sum_d silu(z_d) @ (W2/12 @ U1b)     (12 matmuls, PSUM accum)
  so the "agg" tensor is never materialized except at the two boundary
  chunks (count fixup).  LayerNorm stats are per-token (channel dim on
  partitions) via band-select ones matmuls packed into one PSUM bank in
  two half-batches, so normalization of the first half overlaps pass-1
  compute of the second half.

Engine budget per 512-token chunk (targets):
  ACT   silu(12T) 5.4us + silu(s2) 0.7us            -> floor ~6.1us
  DVE   z-build 3.4 + E/D casts 1.4 + x 0.7 + norm  -> ~5.5-6.9us
  PE    ~22 matmuls x 215ns (warm clock)            -> ~5us
  GPSIMD x^2 (SBUF only - no PSUM port)             -> ~1.9us
  D_B shifted copy runs as SBUF->SBUF DMA.

Sharding: batch dim B=8 -> one batch element per NeuronCore (no halo).
Host pre/post: transpose h -> [128, N] per core, transpose back after.
"""

import numpy as np

K = 6
HID = 128
N = 16384
B = 8
EPS = 1e-5
T = 512                 # token chunk (matmul + elementwise granularity)
NCH = N // T            # 32 chunks
NHALF = NCH // 2        # stats half-batch
OFF0 = 8                # D_full column of token 0 (even, for fp16 alignment)
NCOL = N + 2 * OFF0     # D_full width

# offsets ordered in 4 stride-2 groups: (even uses D_A, odd uses D_B)
NEG_EVEN = [-6, -4, -2]
NEG_ODD = [-5, -3, -1]
POS_ODD = [1, 3, 5]
POS_EVEN = [2, 4, 6]
SEG_ORDER = NEG_EVEN + NEG_ODD + POS_ODD + POS_EVEN  # 12 segments in Z

_compiled = None


def _build_bass(dt_act):
    import concourse.bacc as bacc
    import concourse.bass as bass
    import concourse.tile as tile
    from concourse import mybir

    f32 = mybir.dt.float32
    DT = dt_act

    nc = bacc.Bacc("TRN2", target_bir_lowering=False, debug=False)

    # ---- DRAM I/O ----
    hT = nc.dram_tensor("hT", [HID, N], DT, kind="ExternalInput")
    coordR = nc.dram_tensor("coordR", [1, N], DT, kind="ExternalInput")
    W1a = nc.dram_tensor("W1a", [HID, HID], DT, kind="ExternalInput")
    W1b = nc.dram_tensor("W1b", [HID, HID], DT, kind="ExternalInput")
    w1c = nc.dram_tensor("w1c", [1, HID], DT, kind="ExternalInput")      # +w1c
    w1cn = nc.dram_tensor("w1cn", [1, HID], DT, kind="ExternalInput")    # -w1c
    W2s = nc.dram_tensor("W2s", [HID, HID], DT, kind="ExternalInput")    # W2/12
    W2u = nc.dram_tensor("W2u", [HID, HID], DT, kind="ExternalInput")    # W2/12@U1b
    U1a = nc.dram_tensor("U1a", [HID, HID], DT, kind="ExternalInput")
    U1b = nc.dram_tensor("U1b", [HID, HID], DT, kind="ExternalInput")
    U2 = nc.dram_tensor("U2", [HID, HID], DT, kind="ExternalInput")
    b1c = nc.dram_tensor("b1c", [HID, 1], f32, kind="ExternalInput")      # msg_b1
    buc = nc.dram_tensor("buc", [HID, 1], f32, kind="ExternalInput")      # upd_b1+b2@U1b
    b2uc = nc.dram_tensor("b2uc", [HID, 1], f32, kind="ExternalInput")    # upd_b2 col
    lnbc = nc.dram_tensor("lnbc", [HID, 1], f32, kind="ExternalInput")    # ln_b col
    g_row = nc.dram_tensor("g_row", [1, HID], DT, kind="ExternalInput")   # ln_g
    fixf = nc.dram_tensor("fixf", [1, K], f32, kind="ExternalInput")      # 12/count head
    fixl = nc.dram_tensor("fixl", [1, K], f32, kind="ExternalInput")      # 12/count tail
    # band-select matrix: column 63 = 1/128, else 0 (stats row packing)
    selb = nc.dram_tensor("selb", [HID, 2 * 2 * NCH - 1], DT, kind="ExternalInput")
    outT = nc.dram_tensor("outT", [HID, N], f32, kind="ExternalOutput")

    Silu = mybir.ActivationFunctionType.Silu
    Sqrt = mybir.ActivationFunctionType.Sqrt
    HOT = 2 * NCH - 1   # hot column index in selb

    with tile.TileContext(nc) as tc:
        with (
            tc.tile_pool(name="singles", bufs=1) as singles,
            tc.tile_pool(name="big", bufs=1) as big,
            tc.tile_pool(name="work", bufs=3) as work,
            tc.tile_pool(name="zpool", bufs=3) as zpool,
            tc.tile_pool(name="opool", bufs=3) as opool,
            tc.tile_pool(name="stage", bufs=3) as stpool,
            tc.tile_pool(name="psDE", bufs=2, space="PSUM") as psDE,
            tc.tile_pool(name="psUX", bufs=2, space="PSUM") as psUX,
            tc.tile_pool(name="psPP", bufs=2, space="PSUM") as psPP,
            tc.tile_pool(name="psS", bufs=1, space="PSUM") as psS,
        ):
            # ---- constants into SBUF ----
            sW1a = singles.tile([HID, HID], DT)
            sW1b = singles.tile([HID, HID], DT)
            sW2s = singles.tile([HID, HID], DT)
            sW2u = singles.tile([HID, HID], DT)
            sU1a = singles.tile([HID, HID], DT)
            sU1b = singles.tile([HID, HID], DT)
            sU2 = singles.tile([HID, HID], DT)
            for sb, dr in [(sW1a, W1a), (sW1b, W1b), (sW2s, W2s), (sW2u, W2u),
                           (sU1a, U1a), (sU1b, U1b), (sU2, U2)]:
                nc.sync.dma_start(out=sb, in_=dr[:, :])
            sw1c = singles.tile([1, HID], DT)
            sw1cn = singles.tile([1, HID], DT)
            sg = singles.tile([1, HID], DT)
            for sb, dr in [(sw1c, w1c), (sw1cn, w1cn), (sg, g_row)]:
                nc.sync.dma_start(out=sb, in_=dr[:, :])
            sb1 = singles.tile([HID, 1], f32)
            sbu = singles.tile([HID, 1], f32)
            sb2u = singles.tile([HID, 1], f32)
            slnb = singles.tile([HID, 1], f32)
            for sb, dr in [(sb1, b1c), (sbu, buc), (sb2u, b2uc), (slnb, lnbc)]:
                nc.sync.dma_start(out=sb, in_=dr[:, :])
            # broadcast [1,6] -> [128,6] fix tiles
            sfixf = singles.tile([HID, K], f32)
            sfixl = singles.tile([HID, K], f32)

            def bcast_rows(dr):
                a = dr[0:1, :]
                return bass.AP(tensor=a.tensor, offset=a.offset,
                               ap=[[0, HID]] + list(a.ap[1:]))

            nc.gpsimd.dma_start(out=sfixf, in_=bcast_rows(fixf))
            nc.gpsimd.dma_start(out=sfixl, in_=bcast_rows(fixl))
            ssel = singles.tile([HID, 2 * 2 * NCH - 1], DT)
            nc.sync.dma_start(out=ssel, in_=selb[:, :])

            # ---- big persistent buffers ----
            D_A = big.tile([HID, NCOL], DT)      # token j at col OFF0 + j
            D_B = big.tile([HID, NCOL], DT)      # token j at col OFF0 + 1 + j
            x_full = big.tile([HID, N], DT)
            # zero halo columns of D so boundary silu stays finite
            nc.vector.memset(D_A[:, 0:OFF0], 0.0)
            nc.vector.memset(D_A[:, OFF0 + N:NCOL], 0.0)
            nc.vector.memset(D_B[:, 0:OFF0 + 1], 0.0)
            nc.vector.memset(D_B[:, OFF0 + 1 + N:NCOL], 0.0)

            # LN stats: one PSUM bank per half (chunks 16h..16h+15), i = c % 16:
            #   E[x]  -> row i       (DVE reads need 32-aligned partition start,
            #   E[x2] -> row 32 + i   so the two groups sit at offsets 0 and 32)
            st0_ps = psS.tile([4 * NHALF, T], f32, tag="st0")
            st1_ps = psS.tile([4 * NHALF, T], f32, tag="st1")
            sts = [st0_ps, st1_ps]

            # r|u rows for the normalize pass: row i = [r (T) | mu*r (T)]
            # (one tile per half so DVE writes start at partition 0)
            ru_sb0 = big.tile([NHALF, 2 * T], DT)
            ru_sb1 = big.tile([NHALF, 2 * T], DT)
            ru_sb = [ru_sb0, ru_sb1]
            seps = singles.tile([NHALF, 1], f32)
            nc.vector.memset(seps, float(EPS))

            hts = {}
            crd = {}
            zs = {}

            def load_chunk(c):
                ht = work.tile([HID, T], DT, tag="ht")
                nc.sync.dma_start(out=ht, in_=hT[:, c * T:(c + 1) * T])
                co = work.tile([1, T], DT, tag="co")
                nc.sync.dma_start(out=co, in_=coordR[:, c * T:(c + 1) * T])
                hts[c] = ht
                crd[c] = co

            def phase_d(c):
                # D chunk = W1b.T @ h  +  w1c x coord   (PSUM accumulate)
                d_ps = psDE.tile([HID, T], f32, tag="de")
                nc.tensor.matmul(d_ps, sW1b, hts[c], start=True, stop=False)
                nc.tensor.matmul(d_ps, sw1c, crd[c], start=False, stop=True)
                col = OFF0 + c * T
                nc.vector.tensor_copy(D_A[:, col:col + T], d_ps)
                # shifted copy for odd-offset alignment: SBUF->SBUF DMA
                nc.sync.dma_start(out=D_B[:, col + 1:col + 1 + T],
                                  in_=D_A[:, col:col + T])

            def phase_e(c):
                # E chunk = W1a.T @ h - w1c x coord
                e_ps = psDE.tile([HID, T], f32, tag="de")
                nc.tensor.matmul(e_ps, sW1a, hts[c], start=True, stop=False)
                nc.tensor.matmul(e_ps, sw1cn, crd[c], start=False, stop=True)
                e_sb = work.tile([HID, T], DT, tag="esb")
                nc.vector.tensor_copy(e_sb, e_ps)
                esbs[c] = e_sb

            def seg_in1(tile_ap, col, n):
                # [128, n, T] AP over D with outer column-stride 2
                s = tile_ap[:, col:col + T]
                return bass.AP(tensor=s.tensor, offset=s.offset,
                               ap=[s.ap[0], [2, n], [1, T]])

            def e_bcast(e_sb, n):
                return bass.AP(tensor=e_sb.tensor, offset=e_sb.offset,
                               ap=[e_sb.ap[0], [0, n], [1, T]])

            esbs = {}
            aps = {}
            us = {}
            s2s = {}

            def zpartA(t):
                # negative-offset half: needs only D chunks <= t (all ready)
                e_sb = esbs[t]
                z = zpool.tile([HID, 12 * T], DT, tag="z")
                zv = z.rearrange("p (s t) -> p s t", t=T)
                base = t * T
                nc.vector.tensor_tensor(
                    out=zv[:, 0:3, :], in0=e_bcast(e_sb, 3),
                    in1=seg_in1(D_A, OFF0 + base - 6, 3), op=mybir.AluOpType.add)
                nc.vector.tensor_tensor(
                    out=zv[:, 3:6, :], in0=e_bcast(e_sb, 3),
                    in1=seg_in1(D_B, OFF0 + 1 + base - 5, 3),
                    op=mybir.AluOpType.add)
                # silu first half (segs 0-5), bias = msg_b1
                nc.scalar.activation(z[:, 0:6 * T], z[:, 0:6 * T], Silu,
                                     bias=sb1, scale=1.0)
                zs[t] = (z, zv)

            def zpartB(t):
                # positive-offset half: needs D chunk t+1 (cast this iter)
                z, zv = zs[t]
                e_sb = esbs.pop(t)
                base = t * T
                nc.vector.tensor_tensor(
                    out=zv[:, 9:12, :], in0=e_bcast(e_sb, 3),
                    in1=seg_in1(D_A, OFF0 + base + 2, 3), op=mybir.AluOpType.add)
                nc.vector.tensor_tensor(
                    out=zv[:, 6:9, :], in0=e_bcast(e_sb, 3),
                    in1=seg_in1(D_B, OFF0 + 1 + base + 1, 3),
                    op=mybir.AluOpType.add)
                # silu second half (segs 6-11)
                nc.scalar.activation(z[:, 6 * T:12 * T], z[:, 6 * T:12 * T],
                                     Silu, bias=sb1, scale=1.0)

            def msgA(t):
                # first 6 message matmuls (needs silu half 0)
                _, zv = zs[t]
                boundary = t == 0 or t == NCH - 1
                if t == 0:
                    for s, d in enumerate(SEG_ORDER):
                        if d < 0:
                            nc.vector.memset(zv[:, s, 0:-d], 0.0)
                if boundary:
                    a_ps = psUX.tile([HID, T], f32, tag="ux")
                    for s in range(6):
                        nc.tensor.matmul(a_ps, sW2s, zv[:, s, :],
                                         start=(s == 0), stop=False)
                    aps[t] = a_ps
                else:
                    u_ps = psUX.tile([HID, T], f32, tag="ux")
                    nc.tensor.matmul(u_ps, sU1a, hts[t], start=True, stop=False)
                    for s in range(6):
                        nc.tensor.matmul(u_ps, sW2u, zv[:, s, :],
                                         start=False, stop=False)
                    us[t] = u_ps

            def msgB(t):
                # last 6 message matmuls (needs silu half 1)
                _, zv = zs.pop(t)
                boundary = t == 0 or t == NCH - 1
                if t == NCH - 1:
                    for s, d in enumerate(SEG_ORDER):
                        if d > 0:
                            nc.vector.memset(zv[:, s, T - d:T], 0.0)
                tgt = aps[t] if boundary else us[t]
                w = sW2s if boundary else sW2u
                for s in range(6, 12):
                    nc.tensor.matmul(tgt, w, zv[:, s, :],
                                     start=False, stop=(s == 11))

            def s2em(t):
                # interior only: silu of update-MLP hidden (between silu halves)
                s2 = work.tile([HID, T], DT, tag="s2")
                nc.scalar.activation(s2, us.pop(t), Silu, bias=sbu, scale=1.0)
                s2s[t] = s2

            def tail(t):
                ht = hts[t]
                boundary = t == 0 or t == NCH - 1
                if boundary:
                    a_ps = aps.pop(t)
                    agg = work.tile([HID, T], DT, tag="agg_sb")
                    nc.vector.tensor_copy(agg, a_ps)
                    if t == 0:
                        nc.vector.tensor_tensor(
                            out=agg[:, 0:K], in0=a_ps[:, 0:K],
                            in1=sfixf, op=mybir.AluOpType.mult)
                    else:
                        nc.vector.tensor_tensor(
                            out=agg[:, T - K:T], in0=a_ps[:, T - K:T],
                            in1=sfixl, op=mybir.AluOpType.mult)
                    u_ps = psUX.tile([HID, T], f32, tag="ux")
                    nc.tensor.matmul(u_ps, sU1a, ht, start=True, stop=False)
                    nc.tensor.matmul(u_ps, sU1b, agg, start=False, stop=True)
                    s2 = work.tile([HID, T], DT, tag="s2")
                    nc.scalar.activation(s2, u_ps, Silu, bias=sbu, scale=1.0)
                else:
                    s2 = s2s.pop(t)

                # x = (U2.T@s2 + b2u) + h   (single fused DVE op)
                x_ps = psUX.tile([HID, T], f32, tag="ux")
                nc.tensor.matmul(x_ps, sU2, s2, start=True, stop=True)
                base = t * T
                x_sb = x_full[:, base:base + T]
                nc.vector.scalar_tensor_tensor(
                    out=x_sb, in0=x_ps, scalar=sb2u, in1=ht,
                    op0=mybir.AluOpType.add, op1=mybir.AluOpType.add)
                x2 = work.tile([HID, T], DT, tag="x2")
                nc.vector.tensor_tensor(out=x2, in0=x_sb, in1=x_sb,
                                        op=mybir.AluOpType.mult)
                # stats rows in the half's own bank: E[x] row i, E[x2] row 32+i
                h_, i_ = t // NHALF, t % NHALF
                st = sts[h_]
                r_e2 = 2 * NHALF + i_
                first = i_ == 0
                last = i_ == NHALF - 1
                nc.tensor.matmul(st[:, :], ssel[:, HOT - i_:HOT - i_ + 4 * NHALF],
                                 x_sb, start=first, stop=False)
                nc.tensor.matmul(st[:, :], ssel[:, HOT - r_e2:HOT - r_e2 + 4 * NHALF],
                                 x2, start=False, stop=last)

            def stats_math(h_):
                # batched per-token LN stats for chunks 16h..16h+15
                ru = ru_sb[h_]
                # E[x] rows 0:16 to SBUF; E[x2] stays in PSUM (rows 32:48 —
                # 32-aligned; PSUM+SB operand bases may differ, SB+SB may not)
                ex_sb = work.tile([NHALF, T], f32, tag="ex")
                nc.vector.tensor_copy(ex_sb, sts[h_][0:NHALF, :])
                t1 = work.tile([NHALF, T], f32, tag="t1")
                nc.vector.tensor_tensor(out=t1, in0=ex_sb, in1=ex_sb,
                                        op=mybir.AluOpType.mult)
                var = work.tile([NHALF, T], f32, tag="var")
                nc.vector.tensor_tensor(
                    out=var, in0=sts[h_][2 * NHALF:3 * NHALF, :], in1=t1,
                    op=mybir.AluOpType.subtract)
                nc.scalar.activation(var, var, Sqrt, bias=seps, scale=1.0)
                with nc.allow_low_precision(reason="rstd rows feed fp16 matmuls"):
                    nc.vector.reciprocal(out=ru[:, 0:T], in_=var)
                nc.vector.tensor_tensor(out=ru[:, T:2 * T], in0=ex_sb,
                                        in1=ru[:, 0:T],
                                        op=mybir.AluOpType.mult)

            p2live = {}

            def pass2_pe(t):
                # normalize chunk t, matmul part: p1 = g x r, p2 = g x mu*r
                ru = stpool.tile([1, 2 * T], DT, tag="ru")
                src = ru_sb[t // NHALF]
                nc.sync.dma_start(out=ru, in_=src[t % NHALF:t % NHALF + 1, :])
                p1 = psPP.tile([HID, T], f32, tag="pp")
                nc.tensor.matmul(p1, sg, ru[0:1, 0:T], start=True, stop=True)
                p2 = psPP.tile([HID, T], f32, tag="pp")
                nc.tensor.matmul(p2, sg, ru[0:1, T:2 * T], start=True, stop=True)
                p2live[t] = (p1, p2)

            def pass2_dve(t):
                # out = x*p1 + lnb - p2
                base = t * T
                p1, p2 = p2live.pop(t)
                o = opool.tile([HID, T], f32, tag="o")
                nc.vector.tensor_tensor(out=o, in0=x_full[:, base:base + T],
                                        in1=p1, op=mybir.AluOpType.mult)
                nc.vector.scalar_tensor_tensor(
                    out=o, in0=o, scalar=slnb, in1=p2,
                    op0=mybir.AluOpType.add, op1=mybir.AluOpType.subtract)
                nc.sync.dma_start(out=outT[:, base:base + T], in_=o)

            # ---------------- fused pipeline ----------------
            # iter c: D(c)/E(c) lead their own iteration (front of each FIFO)
            # so the chunk-(c-1) positive-offset builds + silus never sit
            # behind message matmuls.  ACT cadence per iter:
            #   silu_h1(c-1) | s2(c-1) | silu_h0(c)
            # msgA(c) is emitted last so it drains at the next iter's start.
            p2q = []
            load_chunk(0)
            for c in range(NCH + 1):
                if c < NCH:
                    if c + 1 < NCH:
                        load_chunk(c + 1)
                    phase_d(c)
                    phase_e(c)
                pj = p2q.pop(0) if (p2q and c >= NHALF + 1) else None
                if pj is not None:
                    pass2_pe(pj)
                if c >= 1:
                    zpartB(c - 1)
                    msgB(c - 1)
                    if 0 < c - 1 < NCH - 1:
                        s2em(c - 1)
                if c < NCH:
                    zpartA(c)
                if c >= 1:
                    tail(c - 1)
                    if c - 1 == NHALF - 1:
                        stats_math(0)
                        p2q.extend(range(NHALF))
                if pj is not None:
                    pass2_dve(pj)
                if c < NCH:
                    msgA(c)
            stats_math(1)
            for t in range(NHALF, NCH):
                p2q.append(t)
            while p2q:
                pass2_pe(p2q[0])
                pass2_dve(p2q.pop(0))

    nc.compile()
    return nc


def _get_compiled(dt_name):
    global _compiled
    if _compiled is None:
        from concourse import mybir
        dt = {"bf16": mybir.dt.bfloat16, "fp16": mybir.dt.float16,
              "fp32": mybir.dt.float32}[dt_name]
        _compiled = _build_bass(dt)
    return _compiled


DT_NAME = "fp16"


def _sel_band(act_np):
    sel = np.zeros((HID, 2 * 2 * NCH - 1), dtype=np.float32)
    sel[:, 2 * NCH - 1] = 1.0 / HID
    return sel.astype(act_np)


def kernel(**inputs):
    from concourse.bass_utils import run_bass_kernel_spmd

    h = np.asarray(inputs["h"], dtype=np.float32)
    coord = np.asarray(inputs["coord"], dtype=np.float32)
    msg_w1 = np.asarray(inputs["msg_w1"], dtype=np.float32)
    msg_b1 = np.asarray(inputs["msg_b1"], dtype=np.float32)
    msg_w2 = np.asarray(inputs["msg_w2"], dtype=np.float32)
    msg_b2 = np.asarray(inputs["msg_b2"], dtype=np.float32)
    upd_w1 = np.asarray(inputs["upd_w1"], dtype=np.float32)
    upd_b1 = np.asarray(inputs["upd_b1"], dtype=np.float32)
    upd_w2 = np.asarray(inputs["upd_w2"], dtype=np.float32)
    upd_b2 = np.asarray(inputs["upd_b2"], dtype=np.float32)
    ln_g = np.asarray(inputs["ln_g"], dtype=np.float32)
    ln_b = np.asarray(inputs["ln_b"], dtype=np.float32)

    import ml_dtypes
    act_np = {"bf16": ml_dtypes.bfloat16, "fp16": np.float16,
              "fp32": np.float32}[DT_NAME]

    W1a = msg_w1[:HID]
    W1b = msg_w1[HID:2 * HID]
    w1c = msg_w1[2 * HID]
    U1b_f = upd_w1[HID:2 * HID]
    bias_u = upd_b1 + msg_b2 @ U1b_f
    W2s = msg_w2 / (2.0 * K)
    W2u = W2s @ U1b_f

    idx = np.arange(N)
    count = (np.minimum(idx, K) + np.minimum(N - 1 - idx, K)).astype(np.float32)
    fix = (2.0 * K) / count
    fixf = fix[:K].reshape(1, K).astype(np.float32)
    fixl = fix[N - K:].reshape(1, K).astype(np.float32)

    const = {
        "W1a": np.ascontiguousarray(W1a, dtype=act_np),
        "W1b": np.ascontiguousarray(W1b, dtype=act_np),
        "w1c": np.ascontiguousarray(w1c.reshape(1, HID), dtype=act_np),
        "w1cn": np.ascontiguousarray(-w1c.reshape(1, HID), dtype=act_np),
        "W2s": np.ascontiguousarray(W2s, dtype=act_np),
        "W2u": np.ascontiguousarray(W2u, dtype=act_np),
        "U1a": np.ascontiguousarray(upd_w1[:HID], dtype=act_np),
        "U1b": np.ascontiguousarray(U1b_f, dtype=act_np),
        "U2": np.ascontiguousarray(upd_w2, dtype=act_np),
        "b1c": np.ascontiguousarray(msg_b1.reshape(HID, 1), dtype=np.float32),
        "buc": np.ascontiguousarray(bias_u.reshape(HID, 1), dtype=np.float32),
        "b2uc": np.ascontiguousarray(upd_b2.reshape(HID, 1), dtype=np.float32),
        "lnbc": np.ascontiguousarray(ln_b.reshape(HID, 1), dtype=np.float32),
        "g_row": np.ascontiguousarray(ln_g.reshape(1, HID), dtype=act_np),
        "fixf": fixf,
        "fixl": fixl,
        "selb": _sel_band(act_np),
    }

    in_maps = []
    for b in range(B):
        m = dict(const)
        m["hT"] = np.ascontiguousarray(h[b].T, dtype=act_np)
        m["coordR"] = np.ascontiguousarray(coord[b].reshape(1, N), dtype=act_np)
        in_maps.append(m)

    nc = _get_compiled(DT_NAME)
    res = run_bass_kernel_spmd(nc, in_maps, core_ids=list(range(B)))
    global LAST_RESULTS
    LAST_RESULTS = res
    out = np.stack([np.asarray(res.results[b]["outT"], dtype=np.float32).T
                    for b in range(B)])
    return np.ascontiguousarray(out)


# revision 27
# speedup vs baseline: 1.4839x; 1.0991x over previous
"""Trainium2 Bass kernel for the LocalGNOBlock (windowed GNN message passing).

Math restructuring (vs the naive 12x full MLP evaluations):
  msg first layer is linear over concat([h_i, h_j, dc]):
      z_d[i] = (A - C)[i] + (B + C)[i+d] + b1,  d in {+-1..+-6}
  where A = h @ W1a, B = h @ W1b, C = coord x w1c (rank-1).
  Interior chunks fold the whole message-2nd-layer + U1b product:
      u += sum_d silu(z_d) @ (W2/12 @ U1b)     (12 matmuls, PSUM accum)
  so the "agg" tensor is never materialized except at the two boundary
  chunks (count fixup).  LayerNorm stats are per-token (channel dim on
  partitions) via band-select ones matmuls packed into one PSUM bank in
  two half-batches, so normalization of the first half overlaps pass-1
  compute of the second half.

Engine budget per 512-token chunk (targets):
  ACT   silu(12T) 5.4us + silu(s2) 0.7us            -> floor ~6.1us
  DVE   z-build 3.4 + E/D casts 1.4 + x 0.7 + norm  -> ~5.5-6.9us
  PE    ~22 matmuls x 215ns (warm clock)            -> ~5us
  GPSIMD x^2 (SBUF only - no PSUM port)             -> ~1.9us
  D_B shifted copy runs as SBUF->SBUF DMA.

Sharding: batch dim B=8 -> one batch element per NeuronCore (no halo).
Host pre/post: transpose h -> [128, N] per core, transpose back after.
"""

import numpy as np

K = 6
HID = 128
N = 16384
B = 8
EPS = 1e-5
T = 512                 # token chunk (matmul + elementwise granularity)
NCH = N // T            # 32 chunks
NHALF = NCH // 2        # stats half-batch
OFF0 = 8                # D_full column of token 0 (even, for fp16 alignment)
NCOL = N + 2 * OFF0     # D_full width

# offsets ordered in 4 stride-2 groups: (even uses D_A, odd uses D_B)
NEG_EVEN = [-6, -4, -2]
NEG_ODD = [-5, -3, -1]
POS_ODD = [1, 3, 5]
POS_EVEN = [2, 4, 6]
SEG_ORDER = NEG_EVEN + NEG_ODD + POS_ODD + POS_EVEN  # 12 segments in Z

_compiled = None


def _build_bass(dt_act):
    import concourse.bacc as bacc
    import concourse.bass as bass
    import concourse.tile as tile
    from concourse import mybir

    f32 = mybir.dt.float32
    DT = dt_act

    nc = bacc.Bacc("TRN2", target_bir_lowering=False, debug=False)

    # ---- DRAM I/O ----
    hT = nc.dram_tensor("hT", [HID, N], DT, kind="ExternalInput")
    coordR = nc.dram_tensor("coordR", [1, N], DT, kind="ExternalInput")
    W1a = nc.dram_tensor("W1a", [HID, HID], DT, kind="ExternalInput")
    W1b = nc.dram_tensor("W1b", [HID, HID], DT, kind="ExternalInput")
    w1c = nc.dram_tensor("w1c", [1, HID], DT, kind="ExternalInput")      # +w1c
    w1cn = nc.dram_tensor("w1cn", [1, HID], DT, kind="ExternalInput")    # -w1c
    W2s = nc.dram_tensor("W2s", [HID, HID], DT, kind="ExternalInput")    # W2/12
    W2u = nc.dram_tensor("W2u", [HID, HID], DT, kind="ExternalInput")    # W2/12@U1b
    U1a = nc.dram_tensor("U1a", [HID, HID], DT, kind="ExternalInput")
    U1b = nc.dram_tensor("U1b", [HID, HID], DT, kind="ExternalInput")
    U2 = nc.dram_tensor("U2", [HID, HID], DT, kind="ExternalInput")
    b1c = nc.dram_tensor("b1c", [HID, 1], f32, kind="ExternalInput")      # msg_b1
    buc = nc.dram_tensor("buc", [HID, 1], f32, kind="ExternalInput")      # upd_b1+b2@U1b
    b2uc = nc.dram_tensor("b2uc", [HID, 1], f32, kind="ExternalInput")    # upd_b2 col
    lnbc = nc.dram_tensor("lnbc", [HID, 1], f32, kind="ExternalInput")    # ln_b col
    g_row = nc.dram_tensor("g_row", [1, HID], DT, kind="ExternalInput")   # ln_g
    fixf = nc.dram_tensor("fixf", [1, K], f32, kind="ExternalInput")      # 12/count head
    fixl = nc.dram_tensor("fixl", [1, K], f32, kind="ExternalInput")      # 12/count tail
    # band-select matrix: column 63 = 1/128, else 0 (stats row packing)
    selb = nc.dram_tensor("selb", [HID, 2 * 2 * NCH - 1], DT, kind="ExternalInput")
    outT = nc.dram_tensor("outT", [HID, N], f32, kind="ExternalOutput")

    Silu = mybir.ActivationFunctionType.Silu
    Sqrt = mybir.ActivationFunctionType.Sqrt
    HOT = 2 * NCH - 1   # hot column index in selb

    with tile.TileContext(nc) as tc:
        with (
            tc.tile_pool(name="singles", bufs=1) as singles,
            tc.tile_pool(name="big", bufs=1) as big,
            tc.tile_pool(name="work", bufs=3) as work,
            tc.tile_pool(name="zpool", bufs=3) as zpool,
            tc.tile_pool(name="opool", bufs=3) as opool,
            tc.tile_pool(name="stage", bufs=3) as stpool,
            tc.tile_pool(name="psDE", bufs=2, space="PSUM") as psDE,
            tc.tile_pool(name="psUX", bufs=2, space="PSUM") as psUX,
            tc.tile_pool(name="psPP", bufs=2, space="PSUM") as psPP,
            tc.tile_pool(name="psS", bufs=1, space="PSUM") as psS,
        ):
            # ---- constants into SBUF ----
            sW1a = singles.tile([HID, HID], DT)
            sW1b = singles.tile([HID, HID], DT)
            sW2s = singles.tile([HID, HID], DT)
            sW2u = singles.tile([HID, HID], DT)
            sU1a = singles.tile([HID, HID], DT)
            sU1b = singles.tile([HID, HID], DT)
            sU2 = singles.tile([HID, HID], DT)
            for sb, dr in [(sW1a, W1a), (sW1b, W1b), (sW2s, W2s), (sW2u, W2u),
                           (sU1a, U1a), (sU1b, U1b), (sU2, U2)]:
                nc.sync.dma_start(out=sb, in_=dr[:, :])
            sw1c = singles.tile([1, HID], DT)
            sw1cn = singles.tile([1, HID], DT)
            sg = singles.tile([1, HID], DT)
            for sb, dr in [(sw1c, w1c), (sw1cn, w1cn), (sg, g_row)]:
                nc.sync.dma_start(out=sb, in_=dr[:, :])
            sb1 = singles.tile([HID, 1], f32)
            sbu = singles.tile([HID, 1], f32)
            sb2u = singles.tile([HID, 1], f32)
            slnb = singles.tile([HID, 1], f32)
            for sb, dr in [(sb1, b1c), (sbu, buc), (sb2u, b2uc), (slnb, lnbc)]:
                nc.sync.dma_start(out=sb, in_=dr[:, :])
            # broadcast [1,6] -> [128,6] fix tiles
            sfixf = singles.tile([HID, K], f32)
            sfixl = singles.tile([HID, K], f32)

            def bcast_rows(dr):
                a = dr[0:1, :]
                return bass.AP(tensor=a.tensor, offset=a.offset,
                               ap=[[0, HID]] + list(a.ap[1:]))

            nc.gpsimd.dma_start(out=sfixf, in_=bcast_rows(fixf))
            nc.gpsimd.dma_start(out=sfixl, in_=bcast_rows(fixl))
            ssel = singles.tile([HID, 2 * 2 * NCH - 1], DT)
            nc.sync.dma_start(out=ssel, in_=selb[:, :])

            # ---- big persistent buffers ----
            D_A = big.tile([HID, NCOL], DT)      # token j at col OFF0 + j
            D_B = big.tile([HID, NCOL], DT)      # token j at col OFF0 + 1 + j
            x_full = big.tile([HID, N], DT)
            # zero halo columns of D so boundary silu stays finite
            nc.vector.memset(D_A[:, 0:OFF0], 0.0)
            nc.vector.memset(D_A[:, OFF0 + N:NCOL], 0.0)
            nc.vector.memset(D_B[:, 0:OFF0 + 1], 0.0)
            nc.vector.memset(D_B[:, OFF0 + 1 + N:NCOL], 0.0)

            # LN stats: one PSUM bank per half (chunks 16h..16h+15), i = c % 16:
            #   E[x]  -> row i       (DVE reads need 32-aligned partition start,
            #   E[x2] -> row 32 + i   so the two groups sit at offsets 0 and 32)
            st0_ps = psS.tile([4 * NHALF, T], f32, tag="st0")
            st1_ps = psS.tile([4 * NHALF, T], f32, tag="st1")
            sts = [st0_ps, st1_ps]

            # r|u rows for the normalize pass: row i = [r (T) | mu*r (T)]
            # (one tile per half so DVE writes start at partition 0)
            ru_sb0 = big.tile([NHALF, 2 * T], DT)
            ru_sb1 = big.tile([NHALF, 2 * T], DT)
            ru_sb = [ru_sb0, ru_sb1]
            seps = singles.tile([NHALF, 1], f32)
            nc.vector.memset(seps, float(EPS))

            hts = {}
            crd = {}
            zs = {}

            def load_chunk(c):
                # ht lives from load (iter c-2) to the x-op (iter c+2)
                ht = work.tile([HID, T], DT, tag="ht", bufs=6)
                nc.sync.dma_start(out=ht, in_=hT[:, c * T:(c + 1) * T])
                co = work.tile([1, T], DT, tag="co", bufs=4)
                nc.sync.dma_start(out=co, in_=coordR[:, c * T:(c + 1) * T])
                hts[c] = ht
                crd[c] = co

            def phase_d(c):
                # D chunk = W1b.T @ h  +  w1c x coord   (PSUM accumulate)
                d_ps = psDE.tile([HID, T], f32, tag="de")
                nc.tensor.matmul(d_ps, sW1b, hts[c], start=True, stop=False)
                nc.tensor.matmul(d_ps, sw1c, crd[c], start=False, stop=True)
                col = OFF0 + c * T
                nc.vector.tensor_copy(D_A[:, col:col + T], d_ps)
                # shifted copy for odd-offset alignment: SBUF->SBUF DMA
                nc.sync.dma_start(out=D_B[:, col + 1:col + 1 + T],
                                  in_=D_A[:, col:col + T])

            def phase_e(c):
                # E chunk = W1a.T @ h - w1c x coord
                e_ps = psDE.tile([HID, T], f32, tag="de")
                nc.tensor.matmul(e_ps, sW1a, hts[c], start=True, stop=False)
                nc.tensor.matmul(e_ps, sw1cn, crd[c], start=False, stop=True)
                e_sb = work.tile([HID, T], DT, tag="esb")
                nc.vector.tensor_copy(e_sb, e_ps)
                esbs[c] = e_sb

            def seg_in1(tile_ap, col, n):
                # [128, n, T] AP over D with outer column-stride 2
                s = tile_ap[:, col:col + T]
                return bass.AP(tensor=s.tensor, offset=s.offset,
                               ap=[s.ap[0], [2, n], [1, T]])

            def e_bcast(e_sb, n):
                return bass.AP(tensor=e_sb.tensor, offset=e_sb.offset,
                               ap=[e_sb.ap[0], [0, n], [1, T]])

            esbs = {}
            aps = {}
            us = {}
            s2s = {}

            def zbuild(t):
                # build all 12 segments in two DVE ops, then one 12T silu.
                # D_B covers odd offsets -5..-1,+1..+5 = uniform stride 2;
                # D_A covers -6,-4,-2 and +2,+4,+6 = two stride-2 triples
                # with an outer jump of 8 columns (4D access pattern).
                e_sb = esbs.pop(t)
                z = zpool.tile([HID, 12 * T], DT, tag="z")
                zv = z.rearrange("p (s t) -> p s t", t=T)
                base = t * T
                # segs 3..8 <- D_B odd offsets (one 3D op)
                nc.vector.tensor_tensor(
                    out=zv[:, 3:9, :], in0=e_bcast(e_sb, 6),
                    in1=seg_in1(D_B, OFF0 + 1 + base - 5, 6),
                    op=mybir.AluOpType.add)
                # segs 0-2 and 9-11 <- D_A even offsets (one 4D op)
                da = D_A[:, OFF0 + base - 6:OFF0 + base - 6 + T]
                in1_4d = bass.AP(tensor=da.tensor, offset=da.offset,
                                 ap=[da.ap[0], [8, 2], [2, 3], [1, T]])
                zo = zv[:, 0, :]
                out_4d = bass.AP(tensor=zo.tensor, offset=zo.offset,
                                 ap=[zo.ap[0], [9 * T, 2], [T, 3], [1, T]])
                in0_4d = bass.AP(tensor=e_sb.tensor, offset=e_sb.offset,
                                 ap=[e_sb.ap[0], [0, 2], [0, 3], [1, T]])
                nc.vector.tensor_tensor(out=out_4d, in0=in0_4d, in1=in1_4d,
                                        op=mybir.AluOpType.add)
                # silu over all 12 segments (bias = msg_b1)
                nc.scalar.activation(z, z, Silu, bias=sb1, scale=1.0)
                zs[t] = (z, zv)

            def msgAll(t):
                # all 12 message matmuls (+U1a for interior) in one burst
                _, zv = zs.pop(t)
                boundary = t == 0 or t == NCH - 1
                if t == 0:
                    for s, d in enumerate(SEG_ORDER):
                        if d < 0:
                            nc.vector.memset(zv[:, s, 0:-d], 0.0)
                if t == NCH - 1:
                    for s, d in enumerate(SEG_ORDER):
                        if d > 0:
                            nc.vector.memset(zv[:, s, T - d:T], 0.0)
                if boundary:
                    a_ps = psUX.tile([HID, T], f32, tag="ux")
                    for s in range(12):
                        nc.tensor.matmul(a_ps, sW2s, zv[:, s, :],
                                         start=(s == 0), stop=(s == 11))
                    aps[t] = a_ps
                else:
                    u_ps = psUX.tile([HID, T], f32, tag="ux")
                    nc.tensor.matmul(u_ps, sU1a, hts[t], start=True, stop=False)
                    for s in range(12):
                        nc.tensor.matmul(u_ps, sW2u, zv[:, s, :],
                                         start=False, stop=(s == 11))
                    us[t] = u_ps

            def s2em(t):
                # interior: silu of update-MLP hidden
                s2 = work.tile([HID, T], DT, tag="s2")
                nc.scalar.activation(s2, us.pop(t), Silu, bias=sbu, scale=1.0)
                s2s[t] = s2

            def bfix(t):
                # boundary chunks: explicit agg + count fixup + U1b path
                a_ps = aps.pop(t)
                agg = work.tile([HID, T], DT, tag="agg_sb")
                nc.vector.tensor_copy(agg, a_ps)
                if t == 0:
                    nc.vector.tensor_tensor(
                        out=agg[:, 0:K], in0=a_ps[:, 0:K],
                        in1=sfixf, op=mybir.AluOpType.mult)
                else:
                    nc.vector.tensor_tensor(
                        out=agg[:, T - K:T], in0=a_ps[:, T - K:T],
                        in1=sfixl, op=mybir.AluOpType.mult)
                u_ps = psUX.tile([HID, T], f32, tag="ux")
                nc.tensor.matmul(u_ps, sU1a, hts[t], start=True, stop=False)
                nc.tensor.matmul(u_ps, sU1b, agg, start=False, stop=True)
                s2 = work.tile([HID, T], DT, tag="s2")
                nc.scalar.activation(s2, u_ps, Silu, bias=sbu, scale=1.0)
                s2s[t] = s2

            xps = {}

            def tailA(t):
                # x_psum = U2.T @ s2
                x_ps = psUX.tile([HID, T], f32, tag="ux")
                nc.tensor.matmul(x_ps, sU2, s2s.pop(t), start=True, stop=True)
                xps[t] = x_ps

            def tailB(t):
                ht = hts.pop(t)
                x_ps = xps.pop(t)
                # x = (U2.T@s2 + b2u) + h   (single fused DVE op)
                base = t * T
                x_sb = x_full[:, base:base + T]
                nc.vector.scalar_tensor_tensor(
                    out=x_sb, in0=x_ps, scalar=sb2u, in1=ht,
                    op0=mybir.AluOpType.add, op1=mybir.AluOpType.add)
                x2 = work.tile([HID, T], DT, tag="x2")
                nc.vector.tensor_tensor(out=x2, in0=x_sb, in1=x_sb,
                                        op=mybir.AluOpType.mult)
                # stats rows in the half's own bank: E[x] row i, E[x2] row 32+i
                h_, i_ = t // NHALF, t % NHALF
                st = sts[h_]
                r_e2 = 2 * NHALF + i_
                first = i_ == 0
                last = i_ == NHALF - 1
                nc.tensor.matmul(st[:, :], ssel[:, HOT - i_:HOT - i_ + 4 * NHALF],
                                 x_sb, start=first, stop=False)
                nc.tensor.matmul(st[:, :], ssel[:, HOT - r_e2:HOT - r_e2 + 4 * NHALF],
                                 x2, start=False, stop=last)

            def stats_math(h_):
                # batched per-token LN stats for chunks 16h..16h+15
                ru = ru_sb[h_]
                # E[x] rows 0:16 to SBUF; E[x2] stays in PSUM (rows 32:48 —
                # 32-aligned; PSUM+SB operand bases may differ, SB+SB may not)
                ex_sb = work.tile([NHALF, T], f32, tag="ex")
                nc.vector.tensor_copy(ex_sb, sts[h_][0:NHALF, :])
                t1 = work.tile([NHALF, T], f32, tag="t1")
                nc.vector.tensor_tensor(out=t1, in0=ex_sb, in1=ex_sb,
                                        op=mybir.AluOpType.mult)
                var = work.tile([NHALF, T], f32, tag="var")
                nc.vector.tensor_tensor(
                    out=var, in0=sts[h_][2 * NHALF:3 * NHALF, :], in1=t1,
                    op=mybir.AluOpType.subtract)
                nc.scalar.activation(var, var, Sqrt, bias=seps, scale=1.0)
                with nc.allow_low_precision(reason="rstd rows feed fp16 matmuls"):
                    nc.vector.reciprocal(out=ru[:, 0:T], in_=var)
                nc.vector.tensor_tensor(out=ru[:, T:2 * T], in0=ex_sb,
                                        in1=ru[:, 0:T],
                                        op=mybir.AluOpType.mult)

            p2live = {}

            def pass2_pe(t):
                # normalize chunk t, matmul part: p1 = g x r, p2 = g x mu*r
                ru = stpool.tile([1, 2 * T], DT, tag="ru")
                src = ru_sb[t // NHALF]
                nc.sync.dma_start(out=ru, in_=src[t % NHALF:t % NHALF + 1, :])
                p1 = psPP.tile([HID, T], f32, tag="pp")
                nc.tensor.matmul(p1, sg, ru[0:1, 0:T], start=True, stop=True)
                p2 = psPP.tile([HID, T], f32, tag="pp")
                nc.tensor.matmul(p2, sg, ru[0:1, T:2 * T], start=True, stop=True)
                p2live[t] = (p1, p2)

            def pass2_dve(t):
                # out = x*p1 + lnb - p2
                base = t * T
                p1, p2 = p2live.pop(t)
                o = opool.tile([HID, T], f32, tag="o")
                nc.vector.tensor_tensor(out=o, in0=x_full[:, base:base + T],
                                        in1=p1, op=mybir.AluOpType.mult)
                nc.vector.scalar_tensor_tensor(
                    out=o, in0=o, scalar=slnb, in1=p2,
                    op0=mybir.AluOpType.add, op1=mybir.AluOpType.subtract)
                nc.sync.dma_start(out=outT[:, base:base + T], in_=o)

            # ---------------- fused pipeline ----------------
            # D/E run two chunks ahead of the message matmuls, so at every
            # iteration start each engine's queue head is ready:
            #   PE : D,E(c+1) | U2(c-2) | msgAll(c-1) | p1,p2 | stats(c-2)
            #   DVE: D_A cast, e_sb cast | build x2 | x, x2 | o1, o2
            #   ACT: silu(c) | s2(c-1)
            p2q = []
            load_chunk(0)
            load_chunk(1)
            phase_d(0)
            phase_e(0)
            for c in range(NCH + 2):
                if c + 2 < NCH:
                    load_chunk(c + 2)
                if c + 1 < NCH:
                    phase_d(c + 1)
                    phase_e(c + 1)
                if 2 <= c:
                    tailA(c - 2)
                if 1 <= c <= NCH:
                    msgAll(c - 1)
                pj = p2q.pop(0) if (p2q and c >= NHALF + 2) else None
                if pj is not None:
                    pass2_pe(pj)
                if c < NCH:
                    zbuild(c)
                if 2 <= c:
                    tailB(c - 2)
                if 1 <= c <= NCH:
                    t = c - 1
                    if t == 0 or t == NCH - 1:
                        bfix(t)
                    else:
                        s2em(t)
                if c - 2 == NHALF - 1:
                    stats_math(0)
                    p2q.extend(range(NHALF))
                if pj is not None:
                    pass2_dve(pj)
            stats_math(1)
            for t in range(NHALF, NCH):
                p2q.append(t)
            while p2q:
                pass2_pe(p2q[0])
                pass2_dve(p2q.pop(0))

    nc.compile()
    return nc


def _get_compiled(dt_name):
    global _compiled
    if _compiled is None:
        from concourse import mybir
        dt = {"bf16": mybir.dt.bfloat16, "fp16": mybir.dt.float16,
              "fp32": mybir.dt.float32}[dt_name]
        _compiled = _build_bass(dt)
    return _compiled


DT_NAME = "fp16"


def _sel_band(act_np):
    sel = np.zeros((HID, 2 * 2 * NCH - 1), dtype=np.float32)
    sel[:, 2 * NCH - 1] = 1.0 / HID
    return sel.astype(act_np)


def kernel(**inputs):
    from concourse.bass_utils import run_bass_kernel_spmd

    h = np.asarray(inputs["h"], dtype=np.float32)
    coord = np.asarray(inputs["coord"], dtype=np.float32)
    msg_w1 = np.asarray(inputs["msg_w1"], dtype=np.float32)
    msg_b1 = np.asarray(inputs["msg_b1"], dtype=np.float32)
    msg_w2 = np.asarray(inputs["msg_w2"], dtype=np.float32)
    msg_b2 = np.asarray(inputs["msg_b2"], dtype=np.float32)
    upd_w1 = np.asarray(inputs["upd_w1"], dtype=np.float32)
    upd_b1 = np.asarray(inputs["upd_b1"], dtype=np.float32)
    upd_w2 = np.asarray(inputs["upd_w2"], dtype=np.float32)
    upd_b2 = np.asarray(inputs["upd_b2"], dtype=np.float32)
    ln_g = np.asarray(inputs["ln_g"], dtype=np.float32)
    ln_b = np.asarray(inputs["ln_b"], dtype=np.float32)

    import ml_dtypes
    act_np = {"bf16": ml_dtypes.bfloat16, "fp16": np.float16,
              "fp32": np.float32}[DT_NAME]

    W1a = msg_w1[:HID]
    W1b = msg_w1[HID:2 * HID]
    w1c = msg_w1[2 * HID]
    U1b_f = upd_w1[HID:2 * HID]
    bias_u = upd_b1 + msg_b2 @ U1b_f
    W2s = msg_w2 / (2.0 * K)
    W2u = W2s @ U1b_f

    idx = np.arange(N)
    count = (np.minimum(idx, K) + np.minimum(N - 1 - idx, K)).astype(np.float32)
    fix = (2.0 * K) / count
    fixf = fix[:K].reshape(1, K).astype(np.float32)
    fixl = fix[N - K:].reshape(1, K).astype(np.float32)

    const = {
        "W1a": np.ascontiguousarray(W1a, dtype=act_np),
        "W1b": np.ascontiguousarray(W1b, dtype=act_np),
        "w1c": np.ascontiguousarray(w1c.reshape(1, HID), dtype=act_np),
        "w1cn": np.ascontiguousarray(-w1c.reshape(1, HID), dtype=act_np),
        "W2s": np.ascontiguousarray(W2s, dtype=act_np),
        "W2u": np.ascontiguousarray(W2u, dtype=act_np),
        "U1a": np.ascontiguousarray(upd_w1[:HID], dtype=act_np),
        "U1b": np.ascontiguousarray(U1b_f, dtype=act_np),
        "U2": np.ascontiguousarray(upd_w2, dtype=act_np),
        "b1c": np.ascontiguousarray(msg_b1.reshape(HID, 1), dtype=np.float32),
        "buc": np.ascontiguousarray(bias_u.reshape(HID, 1), dtype=np.float32),
        "b2uc": np.ascontiguousarray(upd_b2.reshape(HID, 1), dtype=np.float32),
        "lnbc": np.ascontiguousarray(ln_b.reshape(HID, 1), dtype=np.float32),
        "g_row": np.ascontiguousarray(ln_g.reshape(1, HID), dtype=act_np),
        "fixf": fixf,
        "fixl": fixl,
        "selb": _sel_band(act_np),
    }

    in_maps = []
    for b in range(B):
        m = dict(const)
        m["hT"] = np.ascontiguousarray(h[b].T, dtype=act_np)
        m["coordR"] = np.ascontiguousarray(coord[b].reshape(1, N), dtype=act_np)
        in_maps.append(m)

    nc = _get_compiled(DT_NAME)
    res = run_bass_kernel_spmd(nc, in_maps, core_ids=list(range(B)))
    global LAST_RESULTS
    LAST_RESULTS = res
    out = np.stack([np.asarray(res.results[b]["outT"], dtype=np.float32).T
                    for b in range(B)])
    return np.ascontiguousarray(out)
